# revision 1
# baseline (speedup 1.0000x reference)
"""Trainium2 Bass kernel for nn_AutoRegressive (LSTM cell, 64 autoregressive steps).

Strategy
--------
Data-parallel over batch: B=2048 split across 8 NeuronCores (256 rows each),
params replicated. On-chip dataflow is *feature-major* ("orientation B"):
activations are [feature_partition, batch_free] tiles, so recurrent matmuls
(lhsT = weights stationary, rhs = activations moving) need no transposes.

Key algebraic fusion: the autoregressive feedback is linear —
    x_t = W_d h_{t-1} + b_d
so it folds into the recurrence:
    gates_t = (W_hh + W_ih W_d) h_{t-1} + (b_ih + b_hh + W_ih b_d)
This removes the x-matmuls from the loop entirely (24 of 174 MMs/step) and
moves the dense output matmul off the critical path (it only feeds the output
DMA). Gate biases are applied per-gate via the activation instruction's
per-partition bias operand.

Matmuls run in bf16 (1 PE cycle/row — 4x the fp32 rate — with exact fp32
PSUM accumulation of bf16 products; the saturating LSTM recurrence keeps the
rounding noise bounded — measured ~4e-3 max rel error over 64 steps, and
weights ship at half the DMA bytes). PSUM gate tiles are [128, 4*256] per
128-feature unit, chunk order [i | g | f | o] so i*g can start after two
activations. Emission is software-pipelined across steps (next step's early
K-tiles and the previous step's dense/output work fill the PE pipeline while
the current step's elementwise chain runs) so the PE never starves.
"""

import sys

sys.path.insert(0, "/opt/trn_rl_repo")

import numpy as np

import concourse.bacc as bacc
import concourse.mybir as mybir
import concourse.tile as tile
from concourse.bass_utils import run_bass_kernel_spmd

UNITS = 768
INPUT_DIM = 96
OUT_STEPS = 64
NCORES = 8
B = 2048
BL = B // NCORES  # 256 batch rows per core
NU = UNITS // 128  # 6 unit tiles
F32 = mybir.dt.float32
BF16 = mybir.dt.bfloat16
Sigmoid = mybir.ActivationFunctionType.Sigmoid
Tanh = mybir.ActivationFunctionType.Tanh
MULT = mybir.AluOpType.mult
ADD = mybir.AluOpType.add

# q-slot -> original gate block (PyTorch order i,f,g,o); we use [i, g, f, o]
GATE_PERM = [0, 2, 1, 3]
Q_I, Q_G, Q_F, Q_O = 0, 1, 2, 3

_prog_cache = {}


def _build_program(steps=OUT_STEPS, repeat=1):
    key = (steps, repeat)
    if key in _prog_cache:
        return _prog_cache[key]

    nc = bacc.Bacc("TRN2", target_bir_lowering=False, debug=False, num_devices=NCORES)
    wx_ext = nc.declare_dram_parameter("wx", [128, 4 * UNITS], BF16, isOutput=False)
    wd_ext = nc.declare_dram_parameter("wd", [128, NU, 128], BF16, isOutput=False)
    bd_ext = nc.declare_dram_parameter("bd", [128, 1], F32, isOutput=False)
    bc_ext = nc.declare_dram_parameter("bc", [128, 4 * NU], F32, isOutput=False)
    x0_ext = nc.declare_dram_parameter("x0", [128, BL], BF16, isOutput=False)
    whb_ext = nc.declare_dram_parameter("whb", [128, 6, 4 * UNITS], BF16, isOutput=False)
    out_ext = nc.declare_dram_parameter(
        "out", [steps, INPUT_DIM, BL], F32, isOutput=True
    )

    with tile.TileContext(nc) as tc:
        with (
            tc.tile_pool(name="const", bufs=1) as const,
            tc.tile_pool(name="state", bufs=2) as state,
            tc.tile_pool(name="work", bufs=3) as work,
            tc.tile_pool(name="psg", bufs=3, space="PSUM") as psg,
            tc.tile_pool(name="psd", bufs=1, space="PSUM") as psd,
        ):
            # order matters: step 0 needs x0+wx immediately; wh K-tiles are
            # consumed in order from step 1 on. One queue — the DMA device
            # serializes anyway, so priority order is everything.
            x0 = const.tile([128, BL], BF16, tag="x0")
            nc.sync.dma_start(x0[:], x0_ext[:])
            bd = const.tile([128, 1], F32, tag="bd")
            nc.sync.dma_start(bd[:], bd_ext[:])
            bc = const.tile([128, 4 * NU], F32, tag="bc")
            nc.sync.dma_start(bc[:], bc_ext[:])
            wx = const.tile([128, 4 * UNITS], BF16, tag="wx")
            nc.sync.dma_start(wx[:], wx_ext[:])

            # ramp the PE to its warm P-state on throwaway matmuls while the
            # weight DMAs stream in (x0 arrives almost immediately)
            warm = psd.tile([128, BL], F32, tag="warm")
            for _ in range(30):
                nc.tensor.matmul(
                    warm[:], x0[:, :128], x0[:], start=True, stop=True,
                    skip_group_check=True,
                )

            # weights stay bf16 end-to-end: halves the DMA bytes, LDWEIGHTS
            # costs nothing, bf16 x bf16 products are exact in fp32 PSUM.
            wd = const.tile([128, NU, 128], BF16, tag="wd")
            nc.sync.dma_start(wd[:], wd_ext[:])
            wh = const.tile([128, NU, 4 * UNITS], BF16, tag="wh")
            for k in range(NU):
                nc.sync.dma_start(wh[:, k], whb_ext[:, k])

            def chunk(q):
                return slice(q * BL, (q + 1) * BL)

            def wslice(u, q):
                m = u * 4 + q
                return slice(m * 128, (m + 1) * 128)

            # ---- per-step emission primitives -------------------------------
            def mm_x0(st):
                """Step-0 gates: x0-only matmuls (bias rides x0's ones-row)."""
                for u in range(NU):
                    ps = psg.tile([128, 4 * BL], F32, tag="g")
                    st["ps"][u] = ps
                    for q in range(4):
                        nc.tensor.matmul(
                            ps[:, chunk(q)], wx[:, wslice(u, q)], x0[:],
                            start=True, stop=True,
                        )

            def mm_k(st, u, ks):
                """Gate matmuls for unit u over K-tiles ks (on h_prev)."""
                if st["ps"][u] is None:
                    st["ps"][u] = psg.tile(
                        [128, 4 * BL], F32, tag="g", name=f"g_{st['t']}_{u}"
                    )
                ps = st["ps"][u]
                h_prev = st["h_prev"]
                for k in ks:  # k-outer: the freshest h tile is needed last
                    for q in range(4):
                        nc.tensor.matmul(
                            ps[:, chunk(q)], wh[:, k, wslice(u, q)], h_prev[:, k],
                            start=(k == 0), stop=(k == NU - 1),
                        )

            def dense(st, u, start=None, stop=None):
                """Dense (output) matmul contribution of unit u."""
                nc.tensor.matmul(
                    st["ps_pred"], wd[:, u], st["h_new"][:, u],
                    start=(u == 0) if start is None else start,
                    stop=(u == NU - 1) if stop is None else stop,
                )

            def finalize(st):
                """pred = ps_pred + b_d -> fp32 tile -> DRAM out[t]."""
                pred = work.tile([INPUT_DIM, BL], F32, tag="pred")
                nc.vector.tensor_scalar_add(pred[:], st["ps_pred"][:INPUT_DIM, :], bd[:INPUT_DIM])
                nc.sync.dma_start(out_ext[st["t"]], pred[:])

            def ew(st, u):
                """Elementwise chain for unit u: activations + c/h update."""
                t = st["t"]
                ps = st["ps"][u]
                st["ps"][u] = None
                g_sb = work.tile([128, 4 * BL], F32, tag="gates")
                for q, func in ((Q_I, Sigmoid), (Q_G, Tanh), (Q_F, Sigmoid), (Q_O, Sigmoid)):
                    bias = 0.0 if t == 0 else bc[:, u * 4 + q : u * 4 + q + 1]
                    nc.scalar.activation(g_sb[:, chunk(q)], ps[:, chunk(q)], func, bias=bias)
                i_, g_, f_, o_ = (g_sb[:, chunk(q)] for q in range(4))
                c_new, c_prev = st["c_new"], st["c_prev"]
                if t == 0:
                    nc.vector.tensor_tensor(c_new[:, u], i_, g_, MULT)
                else:
                    m1 = work.tile([128, BL], F32, tag="m1")
                    nc.vector.tensor_tensor(m1[:], i_, g_, MULT)
                    nc.vector.tensor_tensor(c_new[:, u], f_, c_prev[:, u], MULT)
                    nc.vector.tensor_tensor(c_new[:, u], c_new[:, u], m1[:], ADD)
                tct = work.tile([128, BL], F32, tag="tct")
                nc.scalar.activation(tct[:], c_new[:, u], Tanh)
                nc.vector.tensor_tensor(st["h_new"][:, u], o_, tct[:], MULT)

            # ---- whole-kernel emission --------------------------------------
            # prev[] carries deferred work from step t-1 into step t's PE stream:
            # dense u3..u5 + output finalize (their h's are only ready then).
            prev = None  # state dict of step t-1
            for r in range(repeat):
                for t in range(steps):
                    st = {
                        "t": t,
                        "h_prev": prev["h_new"] if t > 0 else None,
                        "c_prev": prev["c_new"] if t > 0 else None,
                        "h_new": state.tile([128, NU, BL], BF16, tag="h", name=f"h_{t}"),
                        "c_new": state.tile([128, NU, BL], F32, tag="c", name=f"c_{t}"),
                        "ps_pred": psd.tile([128, BL], F32, tag="pred", name=f"pred_{t}"),
                        "ps": [None] * NU,
                    }
                    if t == 0:
                        mm_x0(st)
                        for u in range(NU):
                            ew(st, u)
                        # denses + finalize deferred into step 1's stream
                        prev = st
                        continue

                    # steady state: u0+u1's k5 (which need the freshest h of
                    # t-1) run after ~4.3us of guaranteed-ready work; the
                    # previous step's last dense + output finalize fill the
                    # middle; dense u3/u4 land late when their h exists.
                    mm_k(st, 0, range(5))          # u0 k0-4
                    mm_k(st, 1, range(5))          # u1 k0-4
                    mm_k(st, 0, [5])
                    mm_k(st, 1, [5])
                    ew(st, 0)
                    ew(st, 1)
                    if t == 1:
                        for u in range(NU):
                            dense(prev, u)
                        finalize(prev)
                    else:
                        dense(prev, 4)
                        dense(prev, 5)
                        finalize(prev)
                    if t == steps - 1:
                        # final step: process u4/u5 early so their elementwise
                        # chains finish while u2/u3 matmuls run; all denses at
                        # the end then only wait on the last unit (u3).
                        for u in (4, 5, 2, 3):
                            mm_k(st, u, range(6))
                            ew(st, u)
                        for u in (0, 1, 4, 5, 2):
                            dense(st, u, start=(u == 0), stop=False)
                        dense(st, 3, start=False, stop=True)
                        finalize(st)
                    else:
                        for u in (2, 3, 4, 5):
                            mm_k(st, u, range(6))
                            ew(st, u)
                            if u >= 3:
                                dense(st, u - 3)
                        dense(st, 3)
                    prev = st

    nc.compile()
    _prog_cache[key] = nc
    return nc


def _prep_inputs(inputs, W_ih, W_hh, b_ih, b_hh, W_d, b_d):
    """Host-side prep: fuse the dense feedback into the recurrence, permute
    gate columns to [i,g,f,o] unit-interleaved order, build bias tables."""
    import ml_dtypes

    U, I = UNITS, INPUT_DIM
    W_ih = np.asarray(W_ih, np.float64)
    W_hh = np.asarray(W_hh, np.float64)
    W_d = np.asarray(W_d, np.float64)
    b_ih = np.asarray(b_ih, np.float64)
    b_hh = np.asarray(b_hh, np.float64)
    b_d = np.asarray(b_d, np.float64)

    perm = np.empty(4 * U, dtype=np.int64)
    for u in range(NU):
        for q in range(4):
            m = u * 4 + q
            src = GATE_PERM[q] * U + u * 128
            perm[m * 128 : (m + 1) * 128] = np.arange(src, src + 128)

    # step-0 path: gates = W_ih^T x0 + (b_ih + b_hh)  (bias on x0's ones-row)
    b_sum = (b_ih + b_hh)[perm].astype(np.float32)
    wx = np.zeros((128, 4 * U), dtype=np.float32)
    wx[:I] = W_ih.T[:, perm].astype(np.float32)
    wx[I] = b_sum

    # steady path: gates = (W_hh + W_ih W_d)^T h + (b_ih + b_hh + W_ih b_d)
    W_comb_T = (W_hh + W_ih @ W_d).T[:, perm]  # [U, 4U]
    wh = np.ascontiguousarray(
        W_comb_T.reshape(NU, 128, 4 * U).transpose(1, 0, 2)
    ).astype(np.float32)
    b_comb = (b_ih + b_hh + W_ih @ b_d)[perm]  # [4U]
    # bias table [128, 4*NU]: column u*4+q holds the bias for m-tile (u,q)
    bc = np.ascontiguousarray(b_comb.reshape(4 * NU, 128).T).astype(np.float32)

    wd = np.zeros((128, NU, 128), dtype=np.float32)
    wd[:, :, :I] = W_d.T.reshape(NU, 128, I).transpose(1, 0, 2)
    wd = wd.astype(ml_dtypes.bfloat16)
    bd = np.zeros((128, 1), dtype=np.float32)
    bd[:I, 0] = b_d


    wx = wx.astype(ml_dtypes.bfloat16)
    whb = wh.astype(ml_dtypes.bfloat16)  # weights ship as bf16

    x_last = np.asarray(inputs[:, -1, :], dtype=np.float32)  # [B, I]
    in_maps = []
    for c in range(NCORES):
        x0 = np.zeros((128, BL), dtype=np.float32)
        x0[:I] = x_last[c * BL : (c + 1) * BL].T
        x0[I] = 1.0
        x0 = x0.astype(ml_dtypes.bfloat16)
        in_maps.append(
            {"wx": wx, "whb": whb, "wd": wd, "bd": bd, "bc": bc, "x0": x0}
        )
    return in_maps


def kernel(inputs, W_ih, W_hh, b_ih, b_hh, W_d, b_d):
    in_maps = _prep_inputs(
        np.asarray(inputs), W_ih, W_hh, b_ih, b_hh, W_d, b_d
    )
    nc = _build_program()
    res = run_bass_kernel_spmd(nc, in_maps, core_ids=list(range(NCORES)))
    # per-core out: [steps, I, BL] -> [BL, steps, I]; concat cores on batch
    parts = [np.transpose(res.results[c]["out"], (2, 0, 1)) for c in range(NCORES)]
    return np.ascontiguousarray(np.concatenate(parts, axis=0))



# revision 6
# speedup vs baseline: 1.8404x; 1.8404x over previous
"""Trainium2 Bass kernel for nn_AutoRegressive (LSTM cell, 64 autoregressive steps).

Strategy
--------
Data-parallel over batch: B=2048 split across 8 NeuronCores (256 rows each),
params replicated. Feature-major tiles: [feature_partition, batch_free].

The linear autoregressive feedback folds into the recurrence:
    gates_t = (W_hh + W_ih W_d) h_{t-1} + (b_ih + b_hh + W_ih b_d)
so only the fused recurrent matmul + output dense remain per step.

Speed structure (vs the bf16 baseline at 1.04ms):
1. fp8-e4m3 DoubleRow matmuls for the recurrence: each PE pass contracts TWO
   128-row K-slices at 0.5 cycles per moving row -> 4x the bf16 matmul rate.
   W scaled by 2^-11, h by 2^7 (powers of two, exact); the sigma activation's
   scale operand applies s = 2^-18 for free. The saturating LSTM dynamics
   absorb the fp8 noise (measured ~5e-3 rel err vs 3.9e-3 all-bf16).
2. Gate biases are pre-injected into PSUM by one rank-16 bf16 matmul per gate
   tile (chunk-indicator rhs, hi+lo bf16 bias rows), so ONE merged Sigmoid
   instruction covers all 4 gate chunks of a 2-unit group; the g-gate is
   tanh(x) = 2 sigmoid(2x) - 1 with the 2x folded into weights and the affine
   fixup done by a 4x-mode tensor_scalar on DVE.
3. All elementwise tensors are fp16 (not bf16): same 2-byte DVE perf modes
   (2x tensor_tensor / 4x tensor_scalar), 8x lower rounding noise; c stays
   fp32. The dense output matmul runs in fp16 (fp8 there fails the gate).
4. Two independent 128-row batch streams (A/B) are software-pipelined half a
   phase apart: each stream's serial tail (sigma -> c -> tanh -> h -> next
   matmul) hides under the other stream's activation block, so the Activation
   engine (the bottleneck at ~8.3us/step) never starves, and the PE never
   idles >3us (which would drop its p-state in the cost model).
"""

import sys

sys.path.insert(0, "/opt/trn_rl_repo")

import numpy as np

import concourse.bacc as bacc
import concourse.mybir as mybir
import concourse.tile as tile
from concourse.bass_utils import run_bass_kernel_spmd

UNITS = 768
INPUT_DIM = 96
OUT_STEPS = 64
NCORES = 8
B = 2048
BL = B // NCORES          # 256 batch rows per core
SL = BL // 2              # 128 rows per stream
NU = UNITS // 128         # 6 unit tiles
NG = 3                    # 2-unit groups per stream
NJ = NU // 2              # 3 K-pairs for DoubleRow
F32 = mybir.dt.float32
F16 = mybir.dt.float16
BF16 = mybir.dt.bfloat16
F8 = mybir.dt.float8e4
Sigmoid = mybir.ActivationFunctionType.Sigmoid
Tanh = mybir.ActivationFunctionType.Tanh
MULT = mybir.AluOpType.mult
ADD = mybir.AluOpType.add
SUB = mybir.AluOpType.subtract
DR = mybir.MatmulPerfMode.DoubleRow

SW = 2.0 ** -11           # fp8 weight scale
SH = 2.0 ** -7            # fp8 h scale (h8 stores h*128)
S = SW * SH               # sigma scale operand

_prog_cache = {}


def _build_program(steps=OUT_STEPS):
    if steps in _prog_cache:
        return _prog_cache[steps]

    nc = bacc.Bacc("TRN2", target_bir_lowering=False, debug=False, num_devices=NCORES)
    wh_ext = nc.declare_dram_parameter("wh", [128, NJ, 2, 4 * UNITS], F8, isOutput=False)
    wx_ext = nc.declare_dram_parameter("wx", [INPUT_DIM + 1, 4 * UNITS], F16, isOutput=False)
    wb_ext = nc.declare_dram_parameter("wb", [16, NG, 128], BF16, isOutput=False)
    ind_ext = nc.declare_dram_parameter("ind", [16, 8 * SL], BF16, isOutput=False)
    wd_ext = nc.declare_dram_parameter("wd", [128, NU, 128], F16, isOutput=False)
    bd_ext = nc.declare_dram_parameter("bd", [128, 1], F32, isOutput=False)
    x0_ext = nc.declare_dram_parameter("x0", [INPUT_DIM + 1, 2, SL], F16, isOutput=False)
    out_ext = nc.declare_dram_parameter("out", [steps, INPUT_DIM, BL], F32, isOutput=True)

    with tile.TileContext(nc) as tc:
        with (
            tc.tile_pool(name="const", bufs=1) as const,
            tc.tile_pool(name="state", bufs=2) as state,
            tc.tile_pool(name="work", bufs=2) as work,
            tc.tile_pool(name="ps", bufs=4, space="PSUM") as ps,
        ):
            psp = [ps, ps]
            # ---- constants (DMA priority order) ----------------------------
            x0 = const.tile([INPUT_DIM + 1, 2, SL], F16, tag="x0")
            nc.sync.dma_start(x0[:], x0_ext[:])
            ind = const.tile([16, 8 * SL], BF16, tag="ind")
            nc.sync.dma_start(ind[:], ind_ext[:])
            wb = const.tile([16, NG, 128], BF16, tag="wb")
            nc.sync.dma_start(wb[:], wb_ext[:])
            bd = const.tile([128, 1], F32, tag="bd")
            nc.sync.dma_start(bd[:], bd_ext[:])
            wx = const.tile([INPUT_DIM + 1, 4 * UNITS], F16, tag="wx")
            nc.sync.dma_start(wx[:], wx_ext[:])
            wd = const.tile([128, NU, 128], F16, tag="wd")
            nc.sync.dma_start(wd[:], wd_ext[:])
            wh = const.tile([128, NJ, 2, 4 * UNITS], F8, tag="wh")
            for j in range(NJ):
                nc.sync.dma_start(wh[:, j], wh_ext[:, j])

            # ramp the PE to its warm p-state on throwaway matmuls
            warm = ps.tile([128, 8 * SL], F32, tag="g", name="warm")
            for _ in range(16):
                nc.tensor.matmul(
                    warm[:, :256], ind[:, :128], ind[:, :256],
                    start=True, stop=True, skip_group_check=True,
                )

            def chunk(uloc, q):
                return slice((uloc * 4 + q) * SL, (uloc * 4 + q + 1) * SL)

            def new_state(ph, t):
                sfx = f"{'AB'[ph]}{t}"
                return {
                    "t": t, "ph": ph,
                    "gates": [None] * NG,
                    "ps": [None] * NG,
                    "h": state.tile([128, NU, SL], F16, tag=f"h{ph}", name=f"h{sfx}"),
                    "h8": state.tile([128, NU, SL], F8, tag=f"h8{ph}", name=f"h8{sfx}"),
                    "c": state.tile([128, NU, SL], F32, tag=f"c{ph}", name=f"c{sfx}"),
                    "tct": work.tile([128, NU, SL], F16, tag=f"tct{ph}", name=f"tct{sfx}"),
                    "ps_pred": None,
                }

            def emit_gates_pe(st, h8_prev):
                """PE work for one stream's step-t gate tiles, emitted
                per-group [bias, j0, j1, j2] so a stalled bias never blocks
                another group's passes."""
                ph, t = st["ph"], st["t"]
                for g in range(NG):
                    ps = psp[ph].tile(
                        [128, 2, 4, SL], F32, tag="g", name=f"g{'AB'[ph]}{t}_{g}"
                    )
                    st["ps"][g] = ps
                    nc.tensor.matmul(  # rank-16 bias seed (start=True)
                        ps[:], wb[:, g], ind[:],
                        start=True, stop=False, skip_group_check=True,
                    )
                    if t == 0:
                        for uloc in range(2):
                            u = 2 * g + uloc
                            for q in range(4):
                                m = (u * 4 + q) * 128
                                nc.tensor.matmul(
                                    ps[:, uloc, q, :], wx[:, m:m + 128], x0[:, ph],
                                    start=False, stop=(uloc == 1 and q == 3),
                                    skip_group_check=True,
                                )
                    else:
                        for j in range(NJ):
                            for uloc in range(2):
                                u = 2 * g + uloc
                                for q in range(4):
                                    m = (u * 4 + q) * 128
                                    nc.tensor.matmul(
                                        ps[:, uloc, q, :],
                                        wh[:, j, :, m:m + 128],
                                        h8_prev[:, 2 * j:2 * j + 2, :],
                                        start=False, stop=(j == NJ - 1),
                                        perf_mode=DR, skip_group_check=True,
                                    )

            def emit_dense_pe(st):
                ph, t = st["ph"], st["t"]
                ps_pred = psp[ph].tile([128, SL], F32, tag="g", name=f"p{'AB'[ph]}{t}")
                st["ps_pred"] = ps_pred
                for k in range(NU):
                    nc.tensor.matmul(
                        ps_pred[:], wd[:, k], st["h"][:, k],
                        start=(k == 0), stop=(k == NU - 1),
                    )

            def emit_sigma(st, g):
                ph, t = st["ph"], st["t"]
                gates = work.tile(
                    [128, 2, 4, SL], F16, tag=f"gt{ph}", bufs=4,
                    name=f"gt{'AB'[ph]}{t}_{g}",
                )
                st["gates"][g] = gates
                nc.scalar.activation(gates[:], st["ps"][g][:], Sigmoid, scale=S)
                st["ps"][g] = None

            def emit_chain(st, c_prev, g):
                """DVE: g-fix, i*g, f*c; Pool: c = fc + m1."""
                ph, t = st["ph"], st["t"]
                gates = st["gates"][g]
                i_ = gates[:, :, 0, :]
                f_ = gates[:, :, 1, :]
                gp = gates[:, :, 2, :]
                nc.vector.tensor_scalar(gp, gp, 2.0, 1.0, MULT, SUB)
                m1 = work.tile([128, 2, SL], F16, tag=f"m1{ph}", name=f"m1{'AB'[ph]}{t}_{g}")
                nc.vector.tensor_tensor(m1[:], i_, gp, MULT)
                cs = slice(2 * g, 2 * g + 2)
                if t == 0:
                    nc.gpsimd.tensor_scalar_add(st["c"][:, cs], m1[:], 0.0)
                else:
                    fc = work.tile([128, 2, SL], F32, tag=f"fc{ph}", name=f"fc{'AB'[ph]}{t}_{g}")
                    nc.vector.tensor_tensor(fc[:], f_, c_prev[:, cs], MULT)
                    nc.gpsimd.tensor_tensor(st["c"][:, cs], fc[:], m1[:], ADD)

            def emit_tanh(st, r):
                ts = slice(3 * r, 3 * r + 3)
                nc.scalar.activation(st["tct"][:, ts], st["c"][:, ts], Tanh)

            def emit_h(st, g):
                cs = slice(2 * g, 2 * g + 2)
                nc.vector.tensor_tensor(
                    st["h"][:, cs], st["gates"][g][:, :, 3, :], st["tct"][:, cs], MULT
                )
                nc.vector.tensor_scalar(st["h8"][:, cs], st["h"][:, cs], 128.0, None, MULT)

            def emit_finalize(st):
                ph, t = st["ph"], st["t"]
                pred = work.tile([INPUT_DIM, SL], F32, tag=f"pr{ph}", name=f"pr{'AB'[ph]}{t}")
                nc.gpsimd.tensor_scalar_add(pred[:], st["ps_pred"][:INPUT_DIM, :], bd[:INPUT_DIM])
                nc.sync.dma_start(out_ext[t, :, ph * SL:(ph + 1) * SL], pred[:])

            # ---- prologue: stream A step-0 gates ---------------------------
            curA = new_state(0, 0)
            emit_gates_pe(curA, None)
            prevA = prevB = None

            # ---- main loop -------------------------------------------------
            # PE blocks (gates/dense) are emitted AFTER the h-writes they
            # read, in program order; the shared 4-slot psum pool keeps every
            # slot-reuse predecessor at most one phase old.
            for t in range(steps):
                # ======== phase A(t) ========
                cpA = prevA["c"] if t else None
                emit_sigma(curA, 0)
                if t:
                    emit_tanh(prevB, 0)
                emit_chain(curA, cpA, 0)
                if t:
                    emit_h(prevB, 0)
                emit_sigma(curA, 1)
                if t:
                    emit_tanh(prevB, 1)
                emit_chain(curA, cpA, 1)
                if t:
                    emit_h(prevB, 1)
                    emit_h(prevB, 2)
                # hB(t-1) fully written in program order -> B-gates(t)+dense
                curB = new_state(1, t)
                emit_gates_pe(curB, prevB["h8"] if t else None)
                if t:
                    emit_dense_pe(prevB)
                emit_sigma(curA, 2)
                emit_chain(curA, cpA, 2)
                if t:
                    emit_finalize(prevB)

                # ======== phase B(t) ========
                cpB = prevB["c"] if t else None
                emit_sigma(curB, 0)
                emit_tanh(curA, 0)
                emit_chain(curB, cpB, 0)
                emit_h(curA, 0)
                emit_sigma(curB, 1)
                emit_tanh(curA, 1)
                emit_chain(curB, cpB, 1)
                emit_h(curA, 1)
                emit_h(curA, 2)
                # hA(t) fully written -> dense-A(t) + A-gates(t+1)
                emit_dense_pe(curA)
                nxtA = None
                if t < steps - 1:
                    nxtA = new_state(0, t + 1)
                    emit_gates_pe(nxtA, curA["h8"])
                emit_sigma(curB, 2)
                emit_chain(curB, cpB, 2)
                emit_finalize(curA)

                prevA, prevB = curA, curB
                curA = nxtA

            # ---- tail: B(steps-1) tanh/h, dense, finalize ------------------
            emit_tanh(prevB, 0)
            emit_h(prevB, 0)
            emit_tanh(prevB, 1)
            emit_h(prevB, 1)
            emit_h(prevB, 2)
            emit_dense_pe(prevB)
            emit_finalize(prevB)

    nc.compile()
    _prog_cache[steps] = nc
    return nc


def _prep_inputs(inputs, W_ih, W_hh, b_ih, b_hh, W_d, b_d):
    """Host-side prep: fuse dense feedback, chunk-permute, quantize."""
    import ml_dtypes

    U, I = UNITS, INPUT_DIM
    W_ih = np.asarray(W_ih, np.float64)
    W_hh = np.asarray(W_hh, np.float64)
    W_d = np.asarray(W_d, np.float64)
    b_ih = np.asarray(b_ih, np.float64)
    b_hh = np.asarray(b_hh, np.float64)
    b_d = np.asarray(b_d, np.float64)

    W_comb = W_hh + W_ih @ W_d              # [4U, U]
    b_comb = b_ih + b_hh + W_ih @ b_d       # [4U]
    bx = b_ih + b_hh                        # step-0 bias
    db = (bx - b_comb) / S                  # step-0 correction (ones-row)

    # chunk permutation: chunk m=(u,q) <- original gate block q*U + u*128
    perm = np.empty(4 * U, dtype=np.int64)
    for u in range(NU):
        for q in range(4):
            m = u * 4 + q
            src = q * U + u * 128
            perm[m * 128:(m + 1) * 128] = np.arange(src, src + 128)
    Wp = W_comb[perm].astype(np.float64)    # [4U(chunked), U]
    bp = b_comb[perm].copy()
    Wxp = W_ih[perm].copy()                 # [4U, I]
    dbp = db[perm].copy()
    for u in range(NU):                     # double g-gate rows (q==2)
        m = u * 4 + 2
        sl = slice(m * 128, (m + 1) * 128)
        Wp[sl] *= 2.0
        bp[sl] *= 2.0
        Wxp[sl] *= 2.0
        dbp[sl] *= 2.0

    # fp8 DR weights [128, j, i, 4U]: wh[p,j,i,m] = Wp[m, (2j+i)*128+p]/SW
    wh = np.empty((128, NJ, 2, 4 * U), dtype=np.float64)
    for j in range(NJ):
        for i2 in range(2):
            k = 2 * j + i2
            wh[:, j, i2, :] = Wp[:, k * 128:(k + 1) * 128].T / SW
    wh8 = wh.astype(np.float32).astype(ml_dtypes.float8_e4m3)

    # x-path fp16 (pre-divided by S) + bias-correction ones-row
    wxf = np.zeros((I + 1, 4 * U), dtype=np.float64)
    wxf[:I] = (Wxp / S).T
    wxf[I] = dbp
    wx16 = wxf.astype(np.float32).astype(np.float16)

    # rank-16 bias rows (hi+lo bf16) per group
    wbh = np.zeros((16, NG, 128), dtype=np.float64)
    binj = bp / S
    for g in range(NG):
        for uloc in range(2):
            for q in range(4):
                ci = uloc * 4 + q
                m = (2 * g + uloc) * 4 + q
                vals = binj[m * 128:(m + 1) * 128]
                hi = vals.astype(np.float32).astype(ml_dtypes.bfloat16).astype(np.float64)
                wbh[2 * ci, g] = hi
                wbh[2 * ci + 1, g] = vals - hi
    wb16 = wbh.astype(np.float32).astype(ml_dtypes.bfloat16)

    ind = np.zeros((16, 8 * SL), dtype=np.float32)
    for ci in range(8):
        ind[2 * ci, ci * SL:(ci + 1) * SL] = 1.0
        ind[2 * ci + 1, ci * SL:(ci + 1) * SL] = 1.0
    ind16 = ind.astype(ml_dtypes.bfloat16)

    wd = np.zeros((128, NU, 128), dtype=np.float32)
    wd[:, :, :I] = W_d.T.reshape(NU, 128, I).transpose(1, 0, 2)
    wd16 = wd.astype(np.float16)
    bdv = np.zeros((128, 1), dtype=np.float32)
    bdv[:I, 0] = b_d

    x_last = np.asarray(inputs[:, -1, :], dtype=np.float32)  # [B, I]
    in_maps = []
    for c in range(NCORES):
        x0 = np.zeros((I + 1, 2, SL), dtype=np.float32)
        blk = x_last[c * BL:(c + 1) * BL].T   # [I, BL]
        x0[:I, 0] = blk[:, :SL]
        x0[:I, 1] = blk[:, SL:]
        x0[I] = 1.0
        in_maps.append({
            "wh": wh8, "wx": wx16, "wb": wb16, "ind": ind16,
            "wd": wd16, "bd": bdv, "x0": x0.astype(np.float16),
        })
    return in_maps


def kernel(inputs, W_ih, W_hh, b_ih, b_hh, W_d, b_d):
    in_maps = _prep_inputs(np.asarray(inputs), W_ih, W_hh, b_ih, b_hh, W_d, b_d)
    nc = _build_program()
    res = run_bass_kernel_spmd(nc, in_maps, core_ids=list(range(NCORES)))
    parts = [np.transpose(res.results[c]["out"], (2, 0, 1)) for c in range(NCORES)]
    return np.ascontiguousarray(np.concatenate(parts, axis=0))


# revision 8
# speedup vs baseline: 1.8424x; 1.0011x over previous
"""Trainium2 Bass kernel for nn_AutoRegressive (LSTM cell, 64 autoregressive steps).

Strategy
--------
Data-parallel over batch: B=2048 split across 8 NeuronCores (256 rows each),
params replicated. Feature-major tiles: [feature_partition, batch_free].

The linear autoregressive feedback folds into the recurrence:
    gates_t = (W_hh + W_ih W_d) h_{t-1} + (b_ih + b_hh + W_ih b_d)
so only the fused recurrent matmul + output dense remain per step.

Speed structure (vs the bf16 baseline at 1.04ms):
1. fp8-e4m3 DoubleRow matmuls for the recurrence: each PE pass contracts TWO
   128-row K-slices at 0.5 cycles per moving row -> 4x the bf16 matmul rate.
   W scaled by 2^-11, h by 2^7 (powers of two, exact); the sigma activation's
   scale operand applies s = 2^-18 for free. The saturating LSTM dynamics
   absorb the fp8 noise (measured ~5e-3 rel err vs 3.9e-3 all-bf16).
2. Gate biases are pre-injected into PSUM by one rank-16 bf16 matmul per gate
   tile (chunk-indicator rhs, hi+lo bf16 bias rows), so ONE merged Sigmoid
   instruction covers all 4 gate chunks of a 2-unit group; the g-gate is
   tanh(x) = 2 sigmoid(2x) - 1 with the 2x folded into weights and the affine
   fixup done by a 4x-mode tensor_scalar on DVE.
3. All elementwise tensors are fp16 (not bf16): same 2-byte DVE perf modes
   (2x tensor_tensor / 4x tensor_scalar), 8x lower rounding noise; c stays
   fp32. The dense output matmul runs in fp16 (fp8 there fails the gate).
4. Two independent 128-row batch streams (A/B) are software-pipelined half a
   phase apart: each stream's serial tail (sigma -> c -> tanh -> h -> next
   matmul) hides under the other stream's activation block, so the Activation
   engine (the bottleneck at ~8.3us/step) never starves, and the PE never
   idles >3us (which would drop its p-state in the cost model).
"""

import sys

sys.path.insert(0, "/opt/trn_rl_repo")

import numpy as np

import concourse.bacc as bacc
import concourse.mybir as mybir
import concourse.tile as tile
from concourse.bass_utils import run_bass_kernel_spmd

UNITS = 768
INPUT_DIM = 96
OUT_STEPS = 64
NCORES = 8
B = 2048
BL = B // NCORES          # 256 batch rows per core
SL = BL // 2              # 128 rows per stream
NU = UNITS // 128         # 6 unit tiles
NG = 3                    # 2-unit groups per stream
NJ = NU // 2              # 3 K-pairs for DoubleRow
F32 = mybir.dt.float32
F16 = mybir.dt.float16
BF16 = mybir.dt.bfloat16
F8 = mybir.dt.float8e4
Sigmoid = mybir.ActivationFunctionType.Sigmoid
Tanh = mybir.ActivationFunctionType.Tanh
MULT = mybir.AluOpType.mult
ADD = mybir.AluOpType.add
SUB = mybir.AluOpType.subtract
DR = mybir.MatmulPerfMode.DoubleRow

SW = 2.0 ** -11           # fp8 weight scale
SH = 2.0 ** -7            # fp8 h scale (h8 stores h*128)
S = SW * SH               # sigma scale operand

_prog_cache = {}


def _build_program(steps=OUT_STEPS):
    if steps in _prog_cache:
        return _prog_cache[steps]

    nc = bacc.Bacc("TRN2", target_bir_lowering=False, debug=False, num_devices=NCORES)
    wh_ext = nc.declare_dram_parameter("wh", [128, NJ, 2, 4 * UNITS], F8, isOutput=False)
    wx_ext = nc.declare_dram_parameter("wx", [INPUT_DIM + 1, 4 * UNITS], F16, isOutput=False)
    wb_ext = nc.declare_dram_parameter("wb", [16, NG, 128], BF16, isOutput=False)
    ind_ext = nc.declare_dram_parameter("ind", [16, 8 * SL], BF16, isOutput=False)
    wd_ext = nc.declare_dram_parameter("wd", [128, NU, 128], F16, isOutput=False)
    bd_ext = nc.declare_dram_parameter("bd", [128, 1], F32, isOutput=False)
    x0_ext = nc.declare_dram_parameter("x0", [INPUT_DIM + 1, 2, SL], F16, isOutput=False)
    out_ext = nc.declare_dram_parameter("out", [steps, INPUT_DIM, BL], F32, isOutput=True)

    with tile.TileContext(nc) as tc:
        with (
            tc.tile_pool(name="const", bufs=1) as const,
            tc.tile_pool(name="state", bufs=2) as state,
            tc.tile_pool(name="work", bufs=2) as work,
            tc.tile_pool(name="ps", bufs=4, space="PSUM") as ps,
        ):
            psp = [ps, ps]
            # ---- constants (DMA priority order) ----------------------------
            x0 = const.tile([INPUT_DIM + 1, 2, SL], F16, tag="x0")
            nc.sync.dma_start(x0[:], x0_ext[:])
            ind = const.tile([16, 8 * SL], BF16, tag="ind")
            nc.sync.dma_start(ind[:], ind_ext[:])
            wb = const.tile([16, NG, 128], BF16, tag="wb")
            nc.sync.dma_start(wb[:], wb_ext[:])
            bd = const.tile([128, 1], F32, tag="bd")
            nc.sync.dma_start(bd[:], bd_ext[:])
            wx = const.tile([INPUT_DIM + 1, 4 * UNITS], F16, tag="wx")
            nc.sync.dma_start(wx[:], wx_ext[:])
            wd = const.tile([128, NU, 128], F16, tag="wd")
            nc.sync.dma_start(wd[:], wd_ext[:])
            wh = const.tile([128, NJ, 2, 4 * UNITS], F8, tag="wh")
            for j in range(NJ):
                nc.sync.dma_start(wh[:, j], wh_ext[:, j])

            # ramp the PE to its warm p-state on throwaway matmuls
            warm = ps.tile([128, 8 * SL], F32, tag="g", name="warm")
            for _ in range(16):
                nc.tensor.matmul(
                    warm[:, :256], ind[:, :128], ind[:, :256],
                    start=True, stop=True, skip_group_check=True,
                )

            def chunk(uloc, q):
                return slice((uloc * 4 + q) * SL, (uloc * 4 + q + 1) * SL)

            def new_state(ph, t):
                sfx = f"{'AB'[ph]}{t}"
                return {
                    "t": t, "ph": ph,
                    "gates": [None] * NG,
                    "ps": [None] * NG,
                    "h": state.tile([128, NU, SL], F16, tag=f"h{ph}", name=f"h{sfx}"),
                    "h8": state.tile([128, NU, SL], F8, tag=f"h8{ph}", name=f"h8{sfx}"),
                    "c": state.tile([128, NU, SL], F32, tag=f"c{ph}", name=f"c{sfx}"),
                    "tct": work.tile([128, NU, SL], F16, tag=f"tct{ph}", name=f"tct{sfx}"),
                    "ps_pred": None,
                }

            def emit_gates_pe(st, h8_prev):
                """PE work for one stream's step-t gate tiles, emitted
                per-group [bias, j0, j1, j2] so a stalled bias never blocks
                another group's passes."""
                ph, t = st["ph"], st["t"]
                for g in range(NG):
                    ps = psp[ph].tile(
                        [128, 2, 4, SL], F32, tag="g", name=f"g{'AB'[ph]}{t}_{g}"
                    )
                    st["ps"][g] = ps
                    # rank-16 bias seed (start=True); split per uloc-half so
                    # each matmul's output stays within one PSUM bank
                    for uloc in range(2):
                        nc.tensor.matmul(
                            ps[:, uloc], wb[:, g],
                            ind[:, uloc * 4 * SL:(uloc + 1) * 4 * SL],
                            start=True, stop=False, skip_group_check=True,
                        )
                    if t == 0:
                        for uloc in range(2):
                            u = 2 * g + uloc
                            for q in range(4):
                                m = (u * 4 + q) * 128
                                nc.tensor.matmul(
                                    ps[:, uloc, q, :], wx[:, m:m + 128], x0[:, ph],
                                    start=False, stop=(uloc == 1 and q == 3),
                                    skip_group_check=True,
                                )
                    else:
                        for j in range(NJ):
                            for uloc in range(2):
                                u = 2 * g + uloc
                                for q in range(4):
                                    m = (u * 4 + q) * 128
                                    nc.tensor.matmul(
                                        ps[:, uloc, q, :],
                                        wh[:, j, :, m:m + 128],
                                        h8_prev[:, 2 * j:2 * j + 2, :],
                                        start=False, stop=(j == NJ - 1),
                                        perf_mode=DR, skip_group_check=True,
                                    )

            def emit_dense_pe(st):
                ph, t = st["ph"], st["t"]
                ps_pred = psp[ph].tile([128, SL], F32, tag="g", name=f"p{'AB'[ph]}{t}")
                st["ps_pred"] = ps_pred
                for k in range(NU):
                    nc.tensor.matmul(
                        ps_pred[:], wd[:, k], st["h"][:, k],
                        start=(k == 0), stop=(k == NU - 1),
                    )

            def emit_sigma(st, g):
                ph, t = st["ph"], st["t"]
                gates = work.tile(
                    [128, 2, 4, SL], F16, tag=f"gt{ph}", bufs=4,
                    name=f"gt{'AB'[ph]}{t}_{g}",
                )
                st["gates"][g] = gates
                nc.scalar.activation(gates[:], st["ps"][g][:], Sigmoid, scale=S)
                st["ps"][g] = None

            def emit_chain(st, c_prev, g):
                """DVE: g-fix, i*g, f*c; Pool: c = fc + m1."""
                ph, t = st["ph"], st["t"]
                gates = st["gates"][g]
                i_ = gates[:, :, 0, :]
                f_ = gates[:, :, 1, :]
                gp = gates[:, :, 2, :]
                nc.vector.tensor_scalar(gp, gp, 2.0, 1.0, MULT, SUB)
                m1 = work.tile([128, 2, SL], F16, tag=f"m1{ph}", name=f"m1{'AB'[ph]}{t}_{g}")
                nc.vector.tensor_tensor(m1[:], i_, gp, MULT)
                cs = slice(2 * g, 2 * g + 2)
                if t == 0:
                    nc.gpsimd.tensor_scalar_add(st["c"][:, cs], m1[:], 0.0)
                else:
                    fc = work.tile([128, 2, SL], F32, tag=f"fc{ph}", name=f"fc{'AB'[ph]}{t}_{g}")
                    nc.vector.tensor_tensor(fc[:], f_, c_prev[:, cs], MULT)
                    nc.gpsimd.tensor_tensor(st["c"][:, cs], fc[:], m1[:], ADD)

            def emit_tanh(st, r):
                ts = slice(3 * r, 3 * r + 3)
                nc.scalar.activation(st["tct"][:, ts], st["c"][:, ts], Tanh)

            def emit_h(st, g):
                cs = slice(2 * g, 2 * g + 2)
                nc.vector.tensor_tensor(
                    st["h"][:, cs], st["gates"][g][:, :, 3, :], st["tct"][:, cs], MULT
                )
                nc.vector.tensor_scalar(st["h8"][:, cs], st["h"][:, cs], 128.0, None, MULT)

            def emit_finalize(st):
                ph, t = st["ph"], st["t"]
                pred = work.tile([INPUT_DIM, SL], F32, tag=f"pr{ph}", name=f"pr{'AB'[ph]}{t}")
                # GPSIMD has no PSUM port -> this one runs on DVE
                nc.vector.tensor_scalar_add(pred[:], st["ps_pred"][:INPUT_DIM, :], bd[:INPUT_DIM])
                nc.sync.dma_start(out_ext[t, :, ph * SL:(ph + 1) * SL], pred[:])

            # ---- prologue: stream A step-0 gates ---------------------------
            curA = new_state(0, 0)
            emit_gates_pe(curA, None)
            prevA = prevB = None

            # ---- main loop -------------------------------------------------
            # PE blocks (gates/dense) are emitted AFTER the h-writes they
            # read, in program order; the shared 4-slot psum pool keeps every
            # slot-reuse predecessor at most one phase old.
            for t in range(steps):
                # ======== phase A(t) ========
                cpA = prevA["c"] if t else None
                emit_sigma(curA, 0)
                if t:
                    emit_tanh(prevB, 0)
                emit_chain(curA, cpA, 0)
                if t:
                    emit_h(prevB, 0)
                emit_sigma(curA, 1)
                if t:
                    emit_tanh(prevB, 1)
                emit_chain(curA, cpA, 1)
                if t:
                    emit_h(prevB, 1)
                    emit_h(prevB, 2)
                # hB(t-1) fully written in program order -> B-gates(t)+dense
                curB = new_state(1, t)
                emit_gates_pe(curB, prevB["h8"] if t else None)
                if t:
                    emit_dense_pe(prevB)
                emit_sigma(curA, 2)
                emit_chain(curA, cpA, 2)
                if t:
                    emit_finalize(prevB)

                # ======== phase B(t) ========
                cpB = prevB["c"] if t else None
                emit_sigma(curB, 0)
                emit_tanh(curA, 0)
                emit_chain(curB, cpB, 0)
                emit_h(curA, 0)
                emit_sigma(curB, 1)
                emit_tanh(curA, 1)
                emit_chain(curB, cpB, 1)
                emit_h(curA, 1)
                emit_h(curA, 2)
                # hA(t) fully written -> dense-A(t) + A-gates(t+1)
                emit_dense_pe(curA)
                nxtA = None
                if t < steps - 1:
                    nxtA = new_state(0, t + 1)
                    emit_gates_pe(nxtA, curA["h8"])
                emit_sigma(curB, 2)
                emit_chain(curB, cpB, 2)
                emit_finalize(curA)

                prevA, prevB = curA, curB
                curA = nxtA

            # ---- tail: B(steps-1) tanh/h, dense, finalize ------------------
            emit_tanh(prevB, 0)
            emit_h(prevB, 0)
            emit_tanh(prevB, 1)
            emit_h(prevB, 1)
            emit_h(prevB, 2)
            emit_dense_pe(prevB)
            emit_finalize(prevB)

    nc.compile()
    _prog_cache[steps] = nc
    return nc


def _prep_inputs(inputs, W_ih, W_hh, b_ih, b_hh, W_d, b_d):
    """Host-side prep: fuse dense feedback, chunk-permute, quantize."""
    import ml_dtypes

    U, I = UNITS, INPUT_DIM
    W_ih = np.asarray(W_ih, np.float64)
    W_hh = np.asarray(W_hh, np.float64)
    W_d = np.asarray(W_d, np.float64)
    b_ih = np.asarray(b_ih, np.float64)
    b_hh = np.asarray(b_hh, np.float64)
    b_d = np.asarray(b_d, np.float64)

    W_comb = W_hh + W_ih @ W_d              # [4U, U]
    b_comb = b_ih + b_hh + W_ih @ b_d       # [4U]
    bx = b_ih + b_hh                        # step-0 bias
    db = (bx - b_comb) / S                  # step-0 correction (ones-row)

    # chunk permutation: chunk m=(u,q) <- original gate block q*U + u*128
    perm = np.empty(4 * U, dtype=np.int64)
    for u in range(NU):
        for q in range(4):
            m = u * 4 + q
            src = q * U + u * 128
            perm[m * 128:(m + 1) * 128] = np.arange(src, src + 128)
    Wp = W_comb[perm].astype(np.float64)    # [4U(chunked), U]
    bp = b_comb[perm].copy()
    Wxp = W_ih[perm].copy()                 # [4U, I]
    dbp = db[perm].copy()
    for u in range(NU):                     # double g-gate rows (q==2)
        m = u * 4 + 2
        sl = slice(m * 128, (m + 1) * 128)
        Wp[sl] *= 2.0
        bp[sl] *= 2.0
        Wxp[sl] *= 2.0
        dbp[sl] *= 2.0

    # fp8 DR weights [128, j, i, 4U]: wh[p,j,i,m] = Wp[m, (2j+i)*128+p]/SW
    wh = np.empty((128, NJ, 2, 4 * U), dtype=np.float64)
    for j in range(NJ):
        for i2 in range(2):
            k = 2 * j + i2
            wh[:, j, i2, :] = Wp[:, k * 128:(k + 1) * 128].T / SW
    wh8 = wh.astype(np.float32).astype(ml_dtypes.float8_e4m3)

    # x-path fp16 (pre-divided by S) + bias-correction ones-row
    wxf = np.zeros((I + 1, 4 * U), dtype=np.float64)
    wxf[:I] = (Wxp / S).T
    wxf[I] = dbp
    wx16 = wxf.astype(np.float32).astype(np.float16)

    # rank-16 bias rows (hi+lo bf16) per group
    wbh = np.zeros((16, NG, 128), dtype=np.float64)
    binj = bp / S
    for g in range(NG):
        for uloc in range(2):
            for q in range(4):
                ci = uloc * 4 + q
                m = (2 * g + uloc) * 4 + q
                vals = binj[m * 128:(m + 1) * 128]
                hi = vals.astype(np.float32).astype(ml_dtypes.bfloat16).astype(np.float64)
                wbh[2 * ci, g] = hi
                wbh[2 * ci + 1, g] = vals - hi
    wb16 = wbh.astype(np.float32).astype(ml_dtypes.bfloat16)

    ind = np.zeros((16, 8 * SL), dtype=np.float32)
    for ci in range(8):
        ind[2 * ci, ci * SL:(ci + 1) * SL] = 1.0
        ind[2 * ci + 1, ci * SL:(ci + 1) * SL] = 1.0
    ind16 = ind.astype(ml_dtypes.bfloat16)

    wd = np.zeros((128, NU, 128), dtype=np.float32)
    wd[:, :, :I] = W_d.T.reshape(NU, 128, I).transpose(1, 0, 2)
    wd16 = wd.astype(np.float16)
    bdv = np.zeros((128, 1), dtype=np.float32)
    bdv[:I, 0] = b_d

    x_last = np.asarray(inputs[:, -1, :], dtype=np.float32)  # [B, I]
    in_maps = []
    for c in range(NCORES):
        x0 = np.zeros((I + 1, 2, SL), dtype=np.float32)
        blk = x_last[c * BL:(c + 1) * BL].T   # [I, BL]
        x0[:I, 0] = blk[:, :SL]
        x0[:I, 1] = blk[:, SL:]
        x0[I] = 1.0
        in_maps.append({
            "wh": wh8, "wx": wx16, "wb": wb16, "ind": ind16,
            "wd": wd16, "bd": bdv, "x0": x0.astype(np.float16),
        })
    return in_maps


def kernel(inputs, W_ih, W_hh, b_ih, b_hh, W_d, b_d):
    in_maps = _prep_inputs(np.asarray(inputs), W_ih, W_hh, b_ih, b_hh, W_d, b_d)
    nc = _build_program()
    res = run_bass_kernel_spmd(nc, in_maps, core_ids=list(range(NCORES)))
    parts = [np.transpose(res.results[c]["out"], (2, 0, 1)) for c in range(NCORES)]
    return np.ascontiguousarray(np.concatenate(parts, axis=0))


# revision 9
# speedup vs baseline: 1.8494x; 1.0038x over previous
"""Trainium2 Bass kernel for nn_AutoRegressive (LSTM cell, 64 autoregressive steps).

Strategy
--------
Data-parallel over batch: B=2048 split across 8 NeuronCores (256 rows each),
params replicated. Feature-major tiles: [feature_partition, batch_free].

The linear autoregressive feedback folds into the recurrence:
    gates_t = (W_hh + W_ih W_d) h_{t-1} + (b_ih + b_hh + W_ih b_d)
so only the fused recurrent matmul + output dense remain per step.

Speed structure (vs the bf16 baseline at 1.04ms):
1. fp8-e4m3 DoubleRow matmuls for the recurrence: each PE pass contracts TWO
   128-row K-slices at 0.5 cycles per moving row -> 4x the bf16 matmul rate.
   W scaled by 2^-11, h by 2^7 (powers of two, exact); the sigma activation's
   scale operand applies s = 2^-18 for free. The saturating LSTM dynamics
   absorb the fp8 noise (measured ~5e-3 rel err vs 3.9e-3 all-bf16).
2. Gate biases are pre-injected into PSUM by one rank-16 bf16 matmul per gate
   tile (chunk-indicator rhs, hi+lo bf16 bias rows), so ONE merged Sigmoid
   instruction covers all 4 gate chunks of a 2-unit group; the g-gate is
   tanh(x) = 2 sigmoid(2x) - 1 with the 2x folded into weights and the affine
   fixup done by a 4x-mode tensor_scalar on DVE.
3. All elementwise tensors are fp16 (not bf16): same 2-byte DVE perf modes
   (2x tensor_tensor / 4x tensor_scalar), 8x lower rounding noise; c stays
   fp32. The dense output matmul runs in fp16 (fp8 there fails the gate).
4. Two independent 128-row batch streams (A/B) are software-pipelined half a
   phase apart: each stream's serial tail (sigma -> c -> tanh -> h -> next
   matmul) hides under the other stream's activation block, so the Activation
   engine (the bottleneck at ~8.3us/step) never starves, and the PE never
   idles >3us (which would drop its p-state in the cost model).
"""

import sys

sys.path.insert(0, "/opt/trn_rl_repo")

import numpy as np

import concourse.bacc as bacc
import concourse.mybir as mybir
import concourse.tile as tile
from concourse.bass_utils import run_bass_kernel_spmd

UNITS = 768
INPUT_DIM = 96
OUT_STEPS = 64
NCORES = 8
B = 2048
BL = B // NCORES          # 256 batch rows per core
SL = BL // 2              # 128 rows per stream
NU = UNITS // 128         # 6 unit tiles
NG = 3                    # 2-unit groups per stream
NJ = NU // 2              # 3 K-pairs for DoubleRow
F32 = mybir.dt.float32
F16 = mybir.dt.float16
BF16 = mybir.dt.bfloat16
F8 = mybir.dt.float8e4
Sigmoid = mybir.ActivationFunctionType.Sigmoid
Tanh = mybir.ActivationFunctionType.Tanh
MULT = mybir.AluOpType.mult
ADD = mybir.AluOpType.add
SUB = mybir.AluOpType.subtract
DR = mybir.MatmulPerfMode.DoubleRow

SW = 2.0 ** -11           # fp8 weight scale
SH = 2.0 ** -7            # fp8 h scale (h8 stores h*128)
S = SW * SH               # sigma scale operand

_prog_cache = {}


def _build_program(steps=OUT_STEPS):
    if steps in _prog_cache:
        return _prog_cache[steps]

    nc = bacc.Bacc("TRN2", target_bir_lowering=False, debug=False, num_devices=NCORES)
    wh_ext = nc.declare_dram_parameter("wh", [128, NJ, 2, 4 * UNITS], F8, isOutput=False)
    wx_ext = nc.declare_dram_parameter("wx", [INPUT_DIM + 1, 4 * UNITS], F16, isOutput=False)
    wb_ext = nc.declare_dram_parameter("wb", [8, NG, 2, 128], F8, isOutput=False)
    ind_ext = nc.declare_dram_parameter("ind", [8, 2, 8 * SL], F8, isOutput=False)
    wd_ext = nc.declare_dram_parameter("wd", [128, NU, 128], F16, isOutput=False)
    bd_ext = nc.declare_dram_parameter("bd", [128, 1], F32, isOutput=False)
    x0_ext = nc.declare_dram_parameter("x0", [INPUT_DIM + 1, 2, SL], F16, isOutput=False)
    out_ext = nc.declare_dram_parameter("out", [steps, INPUT_DIM, BL], F32, isOutput=True)

    with tile.TileContext(nc) as tc:
        with (
            tc.tile_pool(name="const", bufs=1) as const,
            tc.tile_pool(name="state", bufs=2) as state,
            tc.tile_pool(name="work", bufs=2) as work,
            tc.tile_pool(name="ps", bufs=4, space="PSUM") as ps,
        ):
            psp = [ps, ps]
            # ---- constants (DMA priority order) ----------------------------
            x0 = const.tile([INPUT_DIM + 1, 2, SL], F16, tag="x0")
            nc.sync.dma_start(x0[:], x0_ext[:])
            ind = const.tile([8, 2, 8 * SL], F8, tag="ind")
            nc.sync.dma_start(ind[:], ind_ext[:])
            wb = const.tile([8, NG, 2, 128], F8, tag="wb")
            nc.sync.dma_start(wb[:], wb_ext[:])
            bd = const.tile([128, 1], F32, tag="bd")
            nc.sync.dma_start(bd[:], bd_ext[:])
            wx = const.tile([INPUT_DIM + 1, 4 * UNITS], F16, tag="wx")
            nc.sync.dma_start(wx[:], wx_ext[:])
            wh = const.tile([128, NJ, 2, 4 * UNITS], F8, tag="wh")
            for j in range(NJ):
                nc.sync.dma_start(wh[:, j], wh_ext[:, j])
            wd = const.tile([128, NU, 128], F16, tag="wd")
            nc.sync.dma_start(wd[:], wd_ext[:])

            # ramp the PE to its warm p-state on throwaway matmuls
            warm = ps.tile([128, 8 * SL], F32, tag="g", name="warm")
            for _ in range(16):
                nc.tensor.matmul(
                    warm[:, :256], ind[:, 0, :128], ind[:, 0, :256],
                    start=True, stop=True, skip_group_check=True,
                )

            def chunk(uloc, q):
                return slice((uloc * 4 + q) * SL, (uloc * 4 + q + 1) * SL)

            def new_state(ph, t):
                sfx = f"{'AB'[ph]}{t}"
                return {
                    "t": t, "ph": ph,
                    "gates": [None] * NG,
                    "ps": [None] * NG,
                    "h": state.tile([128, NU, SL], F16, tag=f"h{ph}", name=f"h{sfx}"),
                    "h8": state.tile([128, NU, SL], F8, tag=f"h8{ph}", name=f"h8{sfx}"),
                    "c": state.tile([128, NU, SL], F32, tag=f"c{ph}", name=f"c{sfx}"),
                    "tct": work.tile([128, NU, SL], F16, tag=f"tct{ph}", name=f"tct{sfx}"),
                    "ps_pred": None,
                }

            def emit_gates_pe(st, h8_prev):
                """PE work for one stream's step-t gate tiles, emitted
                per-group [bias, j0, j1, j2] so a stalled bias never blocks
                another group's passes."""
                ph, t = st["ph"], st["t"]
                for g in range(NG):
                    ps = psp[ph].tile(
                        [128, 2, 4, SL], F32, tag="g", name=f"g{'AB'[ph]}{t}_{g}"
                    )
                    st["ps"][g] = ps
                    # rank-8 fp8 DoubleRow bias seed (start=True); split per
                    # uloc-half so each matmul's output stays in one PSUM bank
                    for uloc in range(2):
                        nc.tensor.matmul(
                            ps[:, uloc], wb[:, g],
                            ind[:, :, uloc * 4 * SL:(uloc + 1) * 4 * SL],
                            start=True, stop=False, perf_mode=DR,
                            skip_group_check=True,
                        )
                    if t == 0:
                        for uloc in range(2):
                            u = 2 * g + uloc
                            for q in range(4):
                                m = (u * 4 + q) * 128
                                nc.tensor.matmul(
                                    ps[:, uloc, q, :], wx[:, m:m + 128], x0[:, ph],
                                    start=False, stop=(uloc == 1 and q == 3),
                                    skip_group_check=True,
                                )
                    else:
                        for j in range(NJ):
                            for uloc in range(2):
                                u = 2 * g + uloc
                                for q in range(4):
                                    m = (u * 4 + q) * 128
                                    nc.tensor.matmul(
                                        ps[:, uloc, q, :],
                                        wh[:, j, :, m:m + 128],
                                        h8_prev[:, 2 * j:2 * j + 2, :],
                                        start=False, stop=(j == NJ - 1),
                                        perf_mode=DR, skip_group_check=True,
                                    )

            def emit_dense_pe(st):
                ph, t = st["ph"], st["t"]
                ps_pred = psp[ph].tile([128, SL], F32, tag="g", name=f"p{'AB'[ph]}{t}")
                st["ps_pred"] = ps_pred
                for k in range(NU):
                    nc.tensor.matmul(
                        ps_pred[:], wd[:, k], st["h"][:, k],
                        start=(k == 0), stop=(k == NU - 1),
                    )

            def emit_sigma(st, g):
                ph, t = st["ph"], st["t"]
                gates = work.tile(
                    [128, 2, 4, SL], F16, tag=f"gt{ph}", bufs=4,
                    name=f"gt{'AB'[ph]}{t}_{g}",
                )
                st["gates"][g] = gates
                nc.scalar.activation(gates[:], st["ps"][g][:], Sigmoid, scale=S)
                st["ps"][g] = None

            def emit_chain(st, c_prev, g):
                """DVE: g-fix, i*g, f*c; Pool: c = fc + m1."""
                ph, t = st["ph"], st["t"]
                gates = st["gates"][g]
                i_ = gates[:, :, 0, :]
                f_ = gates[:, :, 1, :]
                gp = gates[:, :, 2, :]
                nc.vector.tensor_scalar(gp, gp, 2.0, 1.0, MULT, SUB)
                m1 = work.tile([128, 2, SL], F16, tag=f"m1{ph}", name=f"m1{'AB'[ph]}{t}_{g}")
                nc.vector.tensor_tensor(m1[:], i_, gp, MULT)
                cs = slice(2 * g, 2 * g + 2)
                if t == 0:
                    nc.gpsimd.tensor_scalar_add(st["c"][:, cs], m1[:], 0.0)
                else:
                    fc = work.tile([128, 2, SL], F32, tag=f"fc{ph}", name=f"fc{'AB'[ph]}{t}_{g}")
                    nc.vector.tensor_tensor(fc[:], f_, c_prev[:, cs], MULT)
                    nc.gpsimd.tensor_tensor(st["c"][:, cs], fc[:], m1[:], ADD)

            def emit_tanh(st, r):
                ts = slice(3 * r, 3 * r + 3)
                nc.scalar.activation(st["tct"][:, ts], st["c"][:, ts], Tanh)

            def emit_h(st, g, last=False):
                cs = slice(2 * g, 2 * g + 2)
                nc.vector.tensor_tensor(
                    st["h"][:, cs], st["gates"][g][:, :, 3, :], st["tct"][:, cs], MULT
                )
                if not last:
                    nc.vector.tensor_scalar(st["h8"][:, cs], st["h"][:, cs], 128.0, None, MULT)

            def emit_finalize(st):
                ph, t = st["ph"], st["t"]
                pred = work.tile([INPUT_DIM, SL], F32, tag=f"pr{ph}", name=f"pr{'AB'[ph]}{t}")
                # GPSIMD has no PSUM port -> this one runs on DVE
                nc.vector.tensor_scalar_add(pred[:], st["ps_pred"][:INPUT_DIM, :], bd[:INPUT_DIM])
                nc.sync.dma_start(out_ext[t, :, ph * SL:(ph + 1) * SL], pred[:])

            # ---- prologue: stream A step-0 gates ---------------------------
            curA = new_state(0, 0)
            emit_gates_pe(curA, None)
            prevA = prevB = None

            # ---- main loop -------------------------------------------------
            # PE blocks (gates/dense) are emitted AFTER the h-writes they
            # read, in program order; the shared 4-slot psum pool keeps every
            # slot-reuse predecessor at most one phase old.
            for t in range(steps):
                # ======== phase A(t) ========
                cpA = prevA["c"] if t else None
                emit_sigma(curA, 0)
                if t:
                    emit_tanh(prevB, 0)
                emit_chain(curA, cpA, 0)
                if t:
                    emit_h(prevB, 0)
                emit_sigma(curA, 1)
                if t:
                    emit_tanh(prevB, 1)
                emit_chain(curA, cpA, 1)
                if t:
                    emit_h(prevB, 1)
                    emit_h(prevB, 2)
                # hB(t-1) fully written in program order -> B-gates(t)+dense
                curB = new_state(1, t)
                emit_gates_pe(curB, prevB["h8"] if t else None)
                if t:
                    emit_dense_pe(prevB)
                emit_sigma(curA, 2)
                emit_chain(curA, cpA, 2)
                if t:
                    emit_finalize(prevB)

                # ======== phase B(t) ========
                cpB = prevB["c"] if t else None
                emit_sigma(curB, 0)
                emit_tanh(curA, 0)
                emit_chain(curB, cpB, 0)
                emit_h(curA, 0, last=(t == steps - 1))
                emit_sigma(curB, 1)
                emit_tanh(curA, 1)
                emit_chain(curB, cpB, 1)
                last = t == steps - 1
                emit_h(curA, 1, last=last)
                emit_h(curA, 2, last=last)
                # hA(t) fully written -> dense-A(t) + A-gates(t+1)
                emit_dense_pe(curA)
                nxtA = None
                if t < steps - 1:
                    nxtA = new_state(0, t + 1)
                    emit_gates_pe(nxtA, curA["h8"])
                emit_sigma(curB, 2)
                emit_chain(curB, cpB, 2)
                emit_finalize(curA)

                prevA, prevB = curA, curB
                curA = nxtA

            # ---- tail: B(steps-1) tanh/h, dense, finalize ------------------
            emit_tanh(prevB, 0)
            emit_h(prevB, 0, last=True)
            emit_tanh(prevB, 1)
            emit_h(prevB, 1, last=True)
            emit_h(prevB, 2, last=True)
            emit_dense_pe(prevB)
            emit_finalize(prevB)

    nc.compile()
    _prog_cache[steps] = nc
    return nc


def _prep_inputs(inputs, W_ih, W_hh, b_ih, b_hh, W_d, b_d):
    """Host-side prep: fuse dense feedback, chunk-permute, quantize."""
    import ml_dtypes

    U, I = UNITS, INPUT_DIM
    W_ih = np.asarray(W_ih, np.float64)
    W_hh = np.asarray(W_hh, np.float64)
    W_d = np.asarray(W_d, np.float64)
    b_ih = np.asarray(b_ih, np.float64)
    b_hh = np.asarray(b_hh, np.float64)
    b_d = np.asarray(b_d, np.float64)

    W_comb = W_hh + W_ih @ W_d              # [4U, U]
    b_comb = b_ih + b_hh + W_ih @ b_d       # [4U]
    bx = b_ih + b_hh                        # step-0 bias
    db = (bx - b_comb) / S                  # step-0 correction (ones-row)

    # chunk permutation: chunk m=(u,q) <- original gate block q*U + u*128
    perm = np.empty(4 * U, dtype=np.int64)
    for u in range(NU):
        for q in range(4):
            m = u * 4 + q
            src = q * U + u * 128
            perm[m * 128:(m + 1) * 128] = np.arange(src, src + 128)
    Wp = W_comb[perm].astype(np.float64)    # [4U(chunked), U]
    bp = b_comb[perm].copy()
    Wxp = W_ih[perm].copy()                 # [4U, I]
    dbp = db[perm].copy()
    for u in range(NU):                     # double g-gate rows (q==2)
        m = u * 4 + 2
        sl = slice(m * 128, (m + 1) * 128)
        Wp[sl] *= 2.0
        bp[sl] *= 2.0
        Wxp[sl] *= 2.0
        dbp[sl] *= 2.0

    # fp8 DR weights [128, j, i, 4U]: wh[p,j,i,m] = Wp[m, (2j+i)*128+p]/SW
    wh = np.empty((128, NJ, 2, 4 * U), dtype=np.float64)
    for j in range(NJ):
        for i2 in range(2):
            k = 2 * j + i2
            wh[:, j, i2, :] = Wp[:, k * 128:(k + 1) * 128].T / SW
    wh8 = wh.astype(np.float32).astype(ml_dtypes.float8_e4m3)

    # x-path fp16 (pre-divided by S) + bias-correction ones-row
    wxf = np.zeros((I + 1, 4 * U), dtype=np.float64)
    wxf[:I] = (Wxp / S).T
    wxf[I] = dbp
    wx16 = wxf.astype(np.float32).astype(np.float16)

    # rank-8 fp8 DoubleRow bias planes: plane0 rows scale 160, plane1 rows
    # scale 10 (second-order residual); both operands fp8-exact powers-ish.
    IND_HI, IND_LO = 160.0, 10.0
    wb8 = np.zeros((8, NG, 2, 128), dtype=np.float64)
    binj = bp / S
    for g in range(NG):
        for uloc in range(2):
            for q in range(4):
                ci = uloc * 4 + q
                m = (2 * g + uloc) * 4 + q
                vals = binj[m * 128:(m + 1) * 128]
                hi = (vals / IND_HI).astype(np.float32).astype(
                    ml_dtypes.float8_e4m3).astype(np.float64)
                lo = ((vals - hi * IND_HI) / IND_LO).astype(np.float32)
                wb8[ci, g, 0] = hi
                wb8[ci, g, 1] = lo
    wb16 = wb8.astype(np.float32).astype(ml_dtypes.float8_e4m3)

    ind = np.zeros((8, 2, 8 * SL), dtype=np.float32)
    for ci in range(8):
        ind[ci, 0, ci * SL:(ci + 1) * SL] = IND_HI
        ind[ci, 1, ci * SL:(ci + 1) * SL] = IND_LO
    ind16 = ind.astype(ml_dtypes.float8_e4m3)

    wd = np.zeros((128, NU, 128), dtype=np.float32)
    wd[:, :, :I] = W_d.T.reshape(NU, 128, I).transpose(1, 0, 2)
    wd16 = wd.astype(np.float16)
    bdv = np.zeros((128, 1), dtype=np.float32)
    bdv[:I, 0] = b_d

    x_last = np.asarray(inputs[:, -1, :], dtype=np.float32)  # [B, I]
    in_maps = []
    for c in range(NCORES):
        x0 = np.zeros((I + 1, 2, SL), dtype=np.float32)
        blk = x_last[c * BL:(c + 1) * BL].T   # [I, BL]
        x0[:I, 0] = blk[:, :SL]
        x0[:I, 1] = blk[:, SL:]
        x0[I] = 1.0
        in_maps.append({
            "wh": wh8, "wx": wx16, "wb": wb16, "ind": ind16,
            "wd": wd16, "bd": bdv, "x0": x0.astype(np.float16),
        })
    return in_maps


def kernel(inputs, W_ih, W_hh, b_ih, b_hh, W_d, b_d):
    in_maps = _prep_inputs(np.asarray(inputs), W_ih, W_hh, b_ih, b_hh, W_d, b_d)
    nc = _build_program()
    res = run_bass_kernel_spmd(nc, in_maps, core_ids=list(range(NCORES)))
    parts = [np.transpose(res.results[c]["out"], (2, 0, 1)) for c in range(NCORES)]
    return np.ascontiguousarray(np.concatenate(parts, axis=0))


# revision 10
# speedup vs baseline: 1.9001x; 1.0274x over previous
"""Trainium2 Bass kernel for nn_AutoRegressive (LSTM cell, 64 autoregressive steps).

Strategy
--------
Data-parallel over batch: B=2048 split across 8 NeuronCores (256 rows each),
params replicated. Feature-major tiles: [feature_partition, batch_free].

The linear autoregressive feedback folds into the recurrence:
    gates_t = (W_hh + W_ih W_d) h_{t-1} + (b_ih + b_hh + W_ih b_d)
so only the fused recurrent matmul + output dense remain per step.

Speed structure (vs the bf16 baseline at 1.04ms):
1. fp8-e4m3 DoubleRow matmuls for the recurrence: each PE pass contracts TWO
   128-row K-slices at 0.5 cycles per moving row -> 4x the bf16 matmul rate.
   W scaled by 2^-11, h by 2^7 (powers of two, exact); the sigma activation's
   scale operand applies s = 2^-18 for free. The saturating LSTM dynamics
   absorb the fp8 noise (measured ~5e-3 rel err vs 3.9e-3 all-bf16).
2. Gate biases are pre-injected into PSUM by tiny rank-4 fp8-DoubleRow
   matmuls (chunk-indicator rhs at scales 160/10, hi+lo fp8 bias rows), so
   ONE merged Sigmoid instruction covers all 12 gate chunks of a 3-unit
   group; the g-gate is tanh(x) = 2 sigmoid(2x) - 1 with the 2x folded into
   the weights and the affine fixup done by a 4x-mode tensor_scalar on DVE.
3. All elementwise tensors are fp16 (not bf16): same 2-byte DVE perf modes
   (2x tensor_tensor / 4x tensor_scalar), 8x lower rounding noise; c stays
   fp32. The dense output matmul runs in fp16 (fp8 there fails the gate).
4. Two independent 128-row batch streams (A/B) are software-pipelined half a
   phase apart: each stream's serial tail (sigma -> c -> tanh -> h -> next
   matmul) hides under the other stream's activation block, so the Activation
   engine (the bottleneck at ~7.9us/step) never starves; tanh instructions
   are scheduled EARLY in the opposite phase so the h-chain feeding the next
   step's matmuls clears before the next sigma needs its PSUM tile. The PE
   never idles >3us (which would drop its p-state in the cost model).
"""

import sys

sys.path.insert(0, "/opt/trn_rl_repo")

import numpy as np

import concourse.bacc as bacc
import concourse.mybir as mybir
import concourse.tile as tile
from concourse.bass_utils import run_bass_kernel_spmd

UNITS = 768
INPUT_DIM = 96
OUT_STEPS = 64
NCORES = 8
B = 2048
BL = B // NCORES          # 256 batch rows per core
SL = BL // 2              # 128 rows per stream
NU = UNITS // 128         # 6 unit tiles
UG = 3                    # units per group
NG = NU // UG             # 2 groups per stream
NJ = NU // 2              # 3 K-pairs for DoubleRow
F32 = mybir.dt.float32
F16 = mybir.dt.float16
BF16 = mybir.dt.bfloat16
F8 = mybir.dt.float8e4
Sigmoid = mybir.ActivationFunctionType.Sigmoid
Tanh = mybir.ActivationFunctionType.Tanh
MULT = mybir.AluOpType.mult
ADD = mybir.AluOpType.add
SUB = mybir.AluOpType.subtract
DR = mybir.MatmulPerfMode.DoubleRow

SW = 2.0 ** -11           # fp8 weight scale
SH = 2.0 ** -7            # fp8 h scale (h8 stores h*128)
S = SW * SH               # sigma scale operand
IND_HI, IND_LO = 160.0, 10.0   # fp8 bias indicator scales

_prog_cache = {}


def _build_program(steps=OUT_STEPS):
    if steps in _prog_cache:
        return _prog_cache[steps]

    nc = bacc.Bacc("TRN2", target_bir_lowering=False, debug=False, num_devices=NCORES)
    wh_ext = nc.declare_dram_parameter("wh", [128, NJ, 2, 4 * UNITS], F8, isOutput=False)
    wx_ext = nc.declare_dram_parameter("wx", [INPUT_DIM + 1, 4 * UNITS], F16, isOutput=False)
    wb_ext = nc.declare_dram_parameter("wb", [4, NU, 2, 128], F8, isOutput=False)
    ind_ext = nc.declare_dram_parameter("ind", [4, 2, 4 * SL], F8, isOutput=False)
    wd_ext = nc.declare_dram_parameter("wd", [128, NU, 128], F16, isOutput=False)
    bd_ext = nc.declare_dram_parameter("bd", [128, 1], F32, isOutput=False)
    x0_ext = nc.declare_dram_parameter("x0", [INPUT_DIM + 1, 2, SL], F16, isOutput=False)
    out_ext = nc.declare_dram_parameter("out", [steps, INPUT_DIM, BL], F32, isOutput=True)

    with tile.TileContext(nc) as tc:
        with (
            tc.tile_pool(name="const", bufs=1) as const,
            tc.tile_pool(name="state", bufs=2) as state,
            tc.tile_pool(name="work", bufs=2) as work,
            tc.tile_pool(name="ps", bufs=2, space="PSUM") as ps,
        ):
            # ---- constants (DMA priority order) ----------------------------
            x0 = const.tile([INPUT_DIM + 1, 2, SL], F16, tag="x0")
            nc.sync.dma_start(x0[:], x0_ext[:])
            ind = const.tile([4, 2, 4 * SL], F8, tag="ind")
            nc.sync.dma_start(ind[:], ind_ext[:])
            wb = const.tile([4, NU, 2, 128], F8, tag="wb")
            nc.sync.dma_start(wb[:], wb_ext[:])
            bd = const.tile([128, 1], F32, tag="bd")
            nc.sync.dma_start(bd[:], bd_ext[:])
            wx = const.tile([INPUT_DIM + 1, 4 * UNITS], F16, tag="wx")
            nc.sync.dma_start(wx[:], wx_ext[:])
            wh = const.tile([128, NJ, 2, 4 * UNITS], F8, tag="wh")
            for j in range(NJ):
                nc.sync.dma_start(wh[:, j], wh_ext[:, j])
            wd = const.tile([128, NU, 128], F16, tag="wd")
            nc.sync.dma_start(wd[:], wd_ext[:])

            # ramp the PE to its warm p-state on throwaway matmuls
            warm = ps.tile([128, 4 * UG * SL], F32, tag="g", name="warm")
            for _ in range(16):
                nc.tensor.matmul(
                    warm[:, :256], ind[:, 0, :128], ind[:, 0, :256],
                    start=True, stop=True, skip_group_check=True,
                )

            def new_state(ph, t):
                sfx = f"{'AB'[ph]}{t}"
                return {
                    "t": t, "ph": ph,
                    "gates": [None] * NG,
                    "ps": [None] * NG,
                    "h": state.tile([128, NU, SL], F16, tag=f"h{ph}", name=f"h{sfx}"),
                    "h8": state.tile([128, NU, SL], F8, tag=f"h8{ph}", name=f"h8{sfx}"),
                    "c": state.tile([128, NU, SL], F32, tag=f"c{ph}", name=f"c{sfx}"),
                    "tct": work.tile([128, NU, SL], F16, tag=f"tct{ph}", name=f"tct{sfx}"),
                    "ps_pred": None,
                }

            def emit_gates_pe(st, h8_prev):
                """PE work for one stream's step-t gate tiles, emitted
                per-group [bias, j0, j1, j2] so a stalled bias never blocks
                another group's passes."""
                ph, t = st["ph"], st["t"]
                for g in range(NG):
                    psg = ps.tile(
                        [128, UG, 4, SL], F32, tag="g", name=f"g{'AB'[ph]}{t}_{g}"
                    )
                    st["ps"][g] = psg
                    # rank-4 fp8 DoubleRow bias seed (start=True); one per
                    # unit-local so each matmul output stays in one PSUM bank
                    for uloc in range(UG):
                        u = UG * g + uloc
                        nc.tensor.matmul(
                            psg[:, uloc], wb[:, u], ind[:],
                            start=True, stop=False, perf_mode=DR,
                            skip_group_check=True,
                        )
                    if t == 0:
                        for uloc in range(UG):
                            u = UG * g + uloc
                            for q in range(4):
                                m = (u * 4 + q) * 128
                                nc.tensor.matmul(
                                    psg[:, uloc, q, :], wx[:, m:m + 128], x0[:, ph],
                                    start=False, stop=(uloc == UG - 1 and q == 3),
                                    skip_group_check=True,
                                )
                    else:
                        for j in range(NJ):
                            for uloc in range(UG):
                                u = UG * g + uloc
                                for q in range(4):
                                    m = (u * 4 + q) * 128
                                    nc.tensor.matmul(
                                        psg[:, uloc, q, :],
                                        wh[:, j, :, m:m + 128],
                                        h8_prev[:, 2 * j:2 * j + 2, :],
                                        start=False, stop=(j == NJ - 1),
                                        perf_mode=DR, skip_group_check=True,
                                    )

            def emit_dense_pe(st):
                ph, t = st["ph"], st["t"]
                ps_pred = ps.tile([128, SL], F32, tag="d", bufs=2, name=f"p{'AB'[ph]}{t}")
                st["ps_pred"] = ps_pred
                for k in range(NU):
                    nc.tensor.matmul(
                        ps_pred[:], wd[:, k], st["h"][:, k],
                        start=(k == 0), stop=(k == NU - 1),
                    )

            def emit_sigma(st, g):
                ph, t = st["ph"], st["t"]
                gates = work.tile(
                    [128, UG, 4, SL], F16, tag=f"gt{ph}", bufs=4,
                    name=f"gt{'AB'[ph]}{t}_{g}",
                )
                st["gates"][g] = gates
                nc.scalar.activation(gates[:], st["ps"][g][:], Sigmoid, scale=S)
                st["ps"][g] = None

            def emit_chain(st, c_prev, g):
                """DVE: g-fix, i*g, f*c; Pool: c = fc + m1."""
                ph, t = st["ph"], st["t"]
                gates = st["gates"][g]
                i_ = gates[:, :, 0, :]
                f_ = gates[:, :, 1, :]
                gp = gates[:, :, 2, :]
                nc.vector.tensor_scalar(gp, gp, 2.0, 1.0, MULT, SUB)
                m1 = work.tile([128, UG, SL], F16, tag=f"m1{ph}", name=f"m1{'AB'[ph]}{t}_{g}")
                nc.vector.tensor_tensor(m1[:], i_, gp, MULT)
                cs = slice(UG * g, UG * (g + 1))
                if t == 0:
                    nc.gpsimd.tensor_scalar_add(st["c"][:, cs], m1[:], 0.0)
                else:
                    fc = work.tile([128, UG, SL], F32, tag=f"fc{ph}", name=f"fc{'AB'[ph]}{t}_{g}")
                    nc.vector.tensor_tensor(fc[:], f_, c_prev[:, cs], MULT)
                    nc.gpsimd.tensor_tensor(st["c"][:, cs], fc[:], m1[:], ADD)

            def emit_tanh(st, g):
                ts = slice(UG * g, UG * (g + 1))
                nc.scalar.activation(st["tct"][:, ts], st["c"][:, ts], Tanh)

            def emit_h(st, g, last=False):
                cs = slice(UG * g, UG * (g + 1))
                nc.vector.tensor_tensor(
                    st["h"][:, cs], st["gates"][g][:, :, 3, :], st["tct"][:, cs], MULT
                )
                if not last:
                    nc.vector.tensor_scalar(st["h8"][:, cs], st["h"][:, cs], 128.0, None, MULT)

            def emit_finalize(st):
                ph, t = st["ph"], st["t"]
                pred = work.tile([INPUT_DIM, SL], F32, tag=f"pr{ph}", name=f"pr{'AB'[ph]}{t}")
                # GPSIMD has no PSUM port -> this one runs on DVE
                nc.vector.tensor_scalar_add(pred[:], st["ps_pred"][:INPUT_DIM, :], bd[:INPUT_DIM])
                nc.sync.dma_start(out_ext[t, :, ph * SL:(ph + 1) * SL], pred[:])

            # ---- prologue: stream A step-0 gates ---------------------------
            curA = new_state(0, 0)
            emit_gates_pe(curA, None)
            prevA = prevB = None

            # ---- main loop -------------------------------------------------
            # Act steady order: [sA0 TB0' TB1' sA1 | sB0 TA0 TA1 sB1]; each
            # stream's tanh pair runs EARLY in the opposite phase so its
            # h-chain clears before the next sigma needs the PSUM tile. PE
            # blocks (gates/dense) are emitted AFTER the h-writes they read.
            for t in range(steps):
                last = t == steps - 1
                # ======== phase A(t) ========
                cpA = prevA["c"] if t else None
                emit_sigma(curA, 0)
                if t:
                    emit_tanh(prevB, 0)
                emit_chain(curA, cpA, 0)
                if t:
                    emit_h(prevB, 0)
                    emit_tanh(prevB, 1)
                    emit_h(prevB, 1)
                emit_sigma(curA, 1)
                emit_chain(curA, cpA, 1)
                # hB(t-1) fully written in program order -> B-gates(t)+dense
                curB = new_state(1, t)
                emit_gates_pe(curB, prevB["h8"] if t else None)
                if t:
                    emit_dense_pe(prevB)
                    emit_finalize(prevB)

                # ======== phase B(t) ========
                cpB = prevB["c"] if t else None
                emit_sigma(curB, 0)
                emit_tanh(curA, 0)
                emit_chain(curB, cpB, 0)
                emit_h(curA, 0, last=last)
                emit_tanh(curA, 1)
                emit_h(curA, 1, last=last)
                emit_sigma(curB, 1)
                emit_chain(curB, cpB, 1)
                # hA(t) fully written -> dense-A(t) + A-gates(t+1)
                emit_dense_pe(curA)
                nxtA = None
                if not last:
                    nxtA = new_state(0, t + 1)
                    emit_gates_pe(nxtA, curA["h8"])
                emit_finalize(curA)

                prevA, prevB = curA, curB
                curA = nxtA

            # ---- tail: B(steps-1) tanh/h, dense, finalize ------------------
            emit_tanh(prevB, 0)
            emit_h(prevB, 0, last=True)
            emit_tanh(prevB, 1)
            emit_h(prevB, 1, last=True)
            emit_dense_pe(prevB)
            emit_finalize(prevB)

    nc.compile()
    _prog_cache[steps] = nc
    return nc


def _prep_inputs(inputs, W_ih, W_hh, b_ih, b_hh, W_d, b_d):
    """Host-side prep: fuse dense feedback, chunk-permute, quantize."""
    import ml_dtypes

    U, I = UNITS, INPUT_DIM
    W_ih = np.asarray(W_ih, np.float64)
    W_hh = np.asarray(W_hh, np.float64)
    W_d = np.asarray(W_d, np.float64)
    b_ih = np.asarray(b_ih, np.float64)
    b_hh = np.asarray(b_hh, np.float64)
    b_d = np.asarray(b_d, np.float64)

    W_comb = W_hh + W_ih @ W_d              # [4U, U]
    b_comb = b_ih + b_hh + W_ih @ b_d       # [4U]
    bx = b_ih + b_hh                        # step-0 bias
    db = (bx - b_comb) / S                  # step-0 correction (ones-row)

    # chunk permutation: chunk m=(u,q) <- original gate block q*U + u*128
    perm = np.empty(4 * U, dtype=np.int64)
    for u in range(NU):
        for q in range(4):
            m = u * 4 + q
            src = q * U + u * 128
            perm[m * 128:(m + 1) * 128] = np.arange(src, src + 128)
    Wp = W_comb[perm].astype(np.float64)    # [4U(chunked), U]
    bp = b_comb[perm].copy()
    Wxp = W_ih[perm].copy()                 # [4U, I]
    dbp = db[perm].copy()
    for u in range(NU):                     # double g-gate rows (q==2)
        m = u * 4 + 2
        sl = slice(m * 128, (m + 1) * 128)
        Wp[sl] *= 2.0
        bp[sl] *= 2.0
        Wxp[sl] *= 2.0
        dbp[sl] *= 2.0

    # fp8 DR weights [128, j, i, 4U]: wh[p,j,i,m] = Wp[m, (2j+i)*128+p]/SW
    wh = np.empty((128, NJ, 2, 4 * U), dtype=np.float64)
    for j in range(NJ):
        for i2 in range(2):
            k = 2 * j + i2
            wh[:, j, i2, :] = Wp[:, k * 128:(k + 1) * 128].T / SW
    wh8 = wh.astype(np.float32).astype(ml_dtypes.float8_e4m3)

    # x-path fp16 (pre-divided by S) + bias-correction ones-row
    wxf = np.zeros((I + 1, 4 * U), dtype=np.float64)
    wxf[:I] = (Wxp / S).T
    wxf[I] = dbp
    wx16 = wxf.astype(np.float32).astype(np.float16)

    # rank-4 fp8 DoubleRow bias rows: plane0 scale 160, plane1 scale 10
    wb8 = np.zeros((4, NU, 2, 128), dtype=np.float64)
    binj = bp / S
    for u in range(NU):
        for q in range(4):
            m = u * 4 + q
            vals = binj[m * 128:(m + 1) * 128]
            hi = (vals / IND_HI).astype(np.float32).astype(
                ml_dtypes.float8_e4m3).astype(np.float64)
            lo = ((vals - hi * IND_HI) / IND_LO).astype(np.float32)
            wb8[q, u, 0] = hi
            wb8[q, u, 1] = lo
    wb8 = wb8.astype(np.float32).astype(ml_dtypes.float8_e4m3)

    ind = np.zeros((4, 2, 4 * SL), dtype=np.float32)
    for q in range(4):
        ind[q, 0, q * SL:(q + 1) * SL] = IND_HI
        ind[q, 1, q * SL:(q + 1) * SL] = IND_LO
    ind8 = ind.astype(ml_dtypes.float8_e4m3)

    wd = np.zeros((128, NU, 128), dtype=np.float32)
    wd[:, :, :I] = W_d.T.reshape(NU, 128, I).transpose(1, 0, 2)
    wd16 = wd.astype(np.float16)
    bdv = np.zeros((128, 1), dtype=np.float32)
    bdv[:I, 0] = b_d

    x_last = np.asarray(inputs[:, -1, :], dtype=np.float32)  # [B, I]
    in_maps = []
    for c in range(NCORES):
        x0 = np.zeros((I + 1, 2, SL), dtype=np.float32)
        blk = x_last[c * BL:(c + 1) * BL].T   # [I, BL]
        x0[:I, 0] = blk[:, :SL]
        x0[:I, 1] = blk[:, SL:]
        x0[I] = 1.0
        in_maps.append({
            "wh": wh8, "wx": wx16, "wb": wb8, "ind": ind8,
            "wd": wd16, "bd": bdv, "x0": x0.astype(np.float16),
        })
    return in_maps


def kernel(inputs, W_ih, W_hh, b_ih, b_hh, W_d, b_d):
    in_maps = _prep_inputs(np.asarray(inputs), W_ih, W_hh, b_ih, b_hh, W_d, b_d)
    nc = _build_program()
    res = run_bass_kernel_spmd(nc, in_maps, core_ids=list(range(NCORES)))
    parts = [np.transpose(res.results[c]["out"], (2, 0, 1)) for c in range(NCORES)]
    return np.ascontiguousarray(np.concatenate(parts, axis=0))


# revision 11
# speedup vs baseline: 1.9242x; 1.0127x over previous
"""Trainium2 Bass kernel for nn_AutoRegressive (LSTM cell, 64 autoregressive steps).

Strategy
--------
Data-parallel over batch: B=2048 split across 8 NeuronCores (256 rows each),
params replicated. Feature-major tiles: [feature_partition, batch_free].

The linear autoregressive feedback folds into the recurrence:
    gates_t = (W_hh + W_ih W_d) h_{t-1} + (b_ih + b_hh + W_ih b_d)
so only the fused recurrent matmul + output dense remain per step.

Speed structure (vs the bf16 baseline at 1.04ms):
1. fp8-e4m3 DoubleRow matmuls for the recurrence: each PE pass contracts TWO
   128-row K-slices at 0.5 cycles per moving row -> 4x the bf16 matmul rate.
   W scaled by 2^-11, h by 2^7 (powers of two, exact); the sigma activation's
   scale operand applies s = 2^-18 for free. The saturating LSTM dynamics
   absorb the fp8 noise (measured ~5e-3 rel err vs 3.9e-3 all-bf16).
2. Gate biases are pre-injected into PSUM by tiny rank-4 fp8-DoubleRow
   matmuls (chunk-indicator rhs at scales 160/10, hi+lo fp8 bias rows), so
   ONE merged Sigmoid instruction covers all 12 gate chunks of a 3-unit
   group; the g-gate is tanh(x) = 2 sigmoid(2x) - 1 with the 2x folded into
   the weights and the affine fixup done by a 4x-mode tensor_scalar on DVE.
3. All elementwise tensors are fp16 (not bf16): same 2-byte DVE perf modes
   (2x tensor_tensor / 4x tensor_scalar), 8x lower rounding noise; c stays
   fp32. The dense output matmul runs in fp16 (fp8 there fails the gate).
4. Two independent 128-row batch streams (A/B) are software-pipelined half a
   phase apart: each stream's serial tail (sigma -> c -> tanh -> h -> next
   matmul) hides under the other stream's activation block, so the Activation
   engine (the bottleneck at ~7.9us/step) never starves; tanh instructions
   are scheduled EARLY in the opposite phase so the h-chain feeding the next
   step's matmuls clears before the next sigma needs its PSUM tile. The PE
   never idles >3us (which would drop its p-state in the cost model).
"""

import sys

sys.path.insert(0, "/opt/trn_rl_repo")

import numpy as np

import concourse.bacc as bacc
import concourse.mybir as mybir
import concourse.tile as tile
from concourse.bass_utils import run_bass_kernel_spmd

UNITS = 768
INPUT_DIM = 96
OUT_STEPS = 64
NCORES = 8
B = 2048
BL = B // NCORES          # 256 batch rows per core
SL = BL // 2              # 128 rows per stream
NU = UNITS // 128         # 6 unit tiles
UG = 3                    # units per group
NG = NU // UG             # 2 groups per stream
NJ = NU // 2              # 3 K-pairs for DoubleRow
F32 = mybir.dt.float32
F16 = mybir.dt.float16
BF16 = mybir.dt.bfloat16
F8 = mybir.dt.float8e4
Sigmoid = mybir.ActivationFunctionType.Sigmoid
Tanh = mybir.ActivationFunctionType.Tanh
MULT = mybir.AluOpType.mult
ADD = mybir.AluOpType.add
SUB = mybir.AluOpType.subtract
DR = mybir.MatmulPerfMode.DoubleRow

SW = 2.0 ** -11           # fp8 weight scale
SH = 2.0 ** -7            # fp8 h scale (h8 stores h*128)
S = SW * SH               # sigma scale operand
IND_HI, IND_LO = 160.0, 10.0   # fp8 bias indicator scales

_prog_cache = {}


def _build_program(steps=OUT_STEPS):
    if steps in _prog_cache:
        return _prog_cache[steps]

    nc = bacc.Bacc("TRN2", target_bir_lowering=False, debug=False, num_devices=NCORES)
    wh_ext = nc.declare_dram_parameter("wh", [128, NJ, 2, 4 * UNITS], F8, isOutput=False)
    wx_ext = nc.declare_dram_parameter("wx", [INPUT_DIM + 1, 4 * UNITS], F16, isOutput=False)
    wb_ext = nc.declare_dram_parameter("wb", [4, NU, 2, 128], F8, isOutput=False)
    ind_ext = nc.declare_dram_parameter("ind", [4, 2, 4 * SL], F8, isOutput=False)
    wd_ext = nc.declare_dram_parameter("wd", [128, NU, 128], F16, isOutput=False)
    bd_ext = nc.declare_dram_parameter("bd", [128, 1], F32, isOutput=False)
    x0_ext = nc.declare_dram_parameter("x0", [INPUT_DIM + 1, 2, SL], F16, isOutput=False)
    out_ext = nc.declare_dram_parameter("out", [steps, INPUT_DIM, BL], F32, isOutput=True)

    with tile.TileContext(nc) as tc:
        with (
            tc.tile_pool(name="const", bufs=1) as const,
            tc.tile_pool(name="state", bufs=2) as state,
            tc.tile_pool(name="work", bufs=2) as work,
            tc.tile_pool(name="ps", bufs=2, space="PSUM") as ps,
        ):
            # ---- constants (DMA priority order) ----------------------------
            x0 = const.tile([INPUT_DIM + 1, 2, SL], F16, tag="x0")
            nc.sync.dma_start(x0[:], x0_ext[:])
            ind = const.tile([4, 2, 4 * SL], F8, tag="ind")
            nc.sync.dma_start(ind[:], ind_ext[:])
            wb = const.tile([4, NU, 2, 128], F8, tag="wb")
            nc.sync.dma_start(wb[:], wb_ext[:])
            bd = const.tile([128, 1], F32, tag="bd")
            nc.sync.dma_start(bd[:], bd_ext[:])
            wx = const.tile([INPUT_DIM + 1, 4 * UNITS], F16, tag="wx")
            nc.sync.dma_start(wx[:], wx_ext[:])
            wh = const.tile([128, NJ, 2, 4 * UNITS], F8, tag="wh")
            for j in range(NJ):
                nc.sync.dma_start(wh[:, j], wh_ext[:, j])
            wd = const.tile([128, NU, 128], F16, tag="wd")
            nc.sync.dma_start(wd[:], wd_ext[:])

            # ramp the PE to its warm p-state on throwaway matmuls
            warm = ps.tile([128, 4 * UG * SL], F32, tag="g", name="warm")
            for _ in range(16):
                nc.tensor.matmul(
                    warm[:, :256], ind[:, 0, :128], ind[:, 0, :256],
                    start=True, stop=True, skip_group_check=True,
                )

            def new_state(ph, t):
                sfx = f"{'AB'[ph]}{t}"
                return {
                    "t": t, "ph": ph,
                    "gates": [None] * NG,
                    "ps": [None] * NG,
                    "h": state.tile([128, NU, SL], F16, tag=f"h{ph}", name=f"h{sfx}"),
                    "h8": state.tile([128, NU, SL], F8, tag=f"h8{ph}", name=f"h8{sfx}"),
                    "c": state.tile([128, NU, SL], F32, tag=f"c{ph}", name=f"c{sfx}"),
                    "tct": work.tile([128, NU, SL], F16, tag=f"tct{ph}", name=f"tct{sfx}"),
                    "ps_pred": None,
                }

            def emit_gates_pe(st, h8_prev):
                """PE work for one stream's step-t gate tiles, emitted
                per-group [bias, j0, j1, j2] so a stalled bias never blocks
                another group's passes."""
                ph, t = st["ph"], st["t"]
                for g in range(NG):
                    psg = ps.tile(
                        [128, UG, 4, SL], F32, tag="g", name=f"g{'AB'[ph]}{t}_{g}"
                    )
                    st["ps"][g] = psg
                    # rank-4 fp8 DoubleRow bias seed (start=True); one per
                    # unit-local so each matmul output stays in one PSUM bank
                    for uloc in range(UG):
                        u = UG * g + uloc
                        nc.tensor.matmul(
                            psg[:, uloc], wb[:, u], ind[:],
                            start=True, stop=False, perf_mode=DR,
                            skip_group_check=True,
                        )
                    if t == 0:
                        for uloc in range(UG):
                            u = UG * g + uloc
                            for q in range(4):
                                m = (u * 4 + q) * 128
                                nc.tensor.matmul(
                                    psg[:, uloc, q, :], wx[:, m:m + 128], x0[:, ph],
                                    start=False, stop=(uloc == UG - 1 and q == 3),
                                    skip_group_check=True,
                                )
                    else:
                        for j in (1, 2, 0):
                            for uloc in range(UG):
                                u = UG * g + uloc
                                for q in range(4):
                                    m = (u * 4 + q) * 128
                                    nc.tensor.matmul(
                                        psg[:, uloc, q, :],
                                        wh[:, j, :, m:m + 128],
                                        h8_prev[:, 2 * j:2 * j + 2, :],
                                        start=False, stop=(j == 0),
                                        perf_mode=DR, skip_group_check=True,
                                    )

            def emit_dense_pe(st):
                ph, t = st["ph"], st["t"]
                ps_pred = ps.tile([128, SL], F32, tag="d", bufs=2, name=f"p{'AB'[ph]}{t}")
                st["ps_pred"] = ps_pred
                for k in range(NU):
                    nc.tensor.matmul(
                        ps_pred[:], wd[:, k], st["h"][:, k],
                        start=(k == 0), stop=(k == NU - 1),
                    )

            def emit_sigma(st, g):
                ph, t = st["ph"], st["t"]
                gates = work.tile(
                    [128, UG, 4, SL], F16, tag=f"gt{ph}", bufs=4,
                    name=f"gt{'AB'[ph]}{t}_{g}",
                )
                st["gates"][g] = gates
                nc.scalar.activation(gates[:], st["ps"][g][:], Sigmoid, scale=S)
                st["ps"][g] = None

            def emit_chain(st, c_prev, g):
                """Pool: f*c (parallel with DVE g-fix/i*g); c-sum on Pool for
                g0, on DVE for g1 (the last group's sum is on the critical
                tanh path, and DVE is faster per element)."""
                ph, t = st["ph"], st["t"]
                gates = st["gates"][g]
                i_ = gates[:, :, 0, :]
                f_ = gates[:, :, 1, :]
                gp = gates[:, :, 2, :]
                nc.vector.tensor_scalar(gp, gp, 2.0, 1.0, MULT, SUB)
                m1 = work.tile([128, UG, SL], F16, tag=f"m1{ph}", name=f"m1{'AB'[ph]}{t}_{g}")
                nc.vector.tensor_tensor(m1[:], i_, gp, MULT)
                cs = slice(UG * g, UG * (g + 1))
                if t == 0:
                    nc.gpsimd.tensor_scalar_add(st["c"][:, cs], m1[:], 0.0)
                else:
                    fc = work.tile([128, UG, SL], F32, tag=f"fc{ph}", name=f"fc{'AB'[ph]}{t}_{g}")
                    nc.gpsimd.tensor_tensor(fc[:], f_, c_prev[:, cs], MULT)
                    eng = nc.vector if g == NG - 1 else nc.gpsimd
                    eng.tensor_tensor(st["c"][:, cs], fc[:], m1[:], ADD)

            def emit_tanh(st, g):
                ts = slice(UG * g, UG * (g + 1))
                nc.scalar.activation(st["tct"][:, ts], st["c"][:, ts], Tanh)

            def emit_h(st, g, last=False):
                cs = slice(UG * g, UG * (g + 1))
                nc.vector.tensor_tensor(
                    st["h"][:, cs], st["gates"][g][:, :, 3, :], st["tct"][:, cs], MULT
                )
                if not last:
                    nc.vector.tensor_scalar(st["h8"][:, cs], st["h"][:, cs], 128.0, None, MULT)

            def emit_finalize(st):
                ph, t = st["ph"], st["t"]
                pred = work.tile([INPUT_DIM, SL], F32, tag=f"pr{ph}", name=f"pr{'AB'[ph]}{t}")
                # GPSIMD has no PSUM port -> this one runs on DVE
                nc.vector.tensor_scalar_add(pred[:], st["ps_pred"][:INPUT_DIM, :], bd[:INPUT_DIM])
                nc.sync.dma_start(out_ext[t, :, ph * SL:(ph + 1) * SL], pred[:])

            # ---- prologue: stream A step-0 gates ---------------------------
            curA = new_state(0, 0)
            emit_gates_pe(curA, None)
            prevA = prevB = None

            # ---- main loop -------------------------------------------------
            # Act steady order: [sA0 TB0' TB1' sA1 | sB0 TA0 TA1 sB1]; each
            # stream's tanh pair runs EARLY in the opposite phase so its
            # h-chain clears before the next sigma needs the PSUM tile. PE
            # blocks (gates/dense) are emitted AFTER the h-writes they read.
            for t in range(steps):
                last = t == steps - 1
                # ======== phase A(t) ========
                cpA = prevA["c"] if t else None
                emit_sigma(curA, 0)
                if t:
                    emit_tanh(prevB, 1)
                    emit_h(prevB, 1)
                    emit_tanh(prevB, 0)
                    emit_h(prevB, 0)
                emit_chain(curA, cpA, 0)
                emit_sigma(curA, 1)
                emit_chain(curA, cpA, 1)
                # hB(t-1) fully written in program order -> B-gates(t)+dense
                curB = new_state(1, t)
                emit_gates_pe(curB, prevB["h8"] if t else None)
                if t:
                    emit_dense_pe(prevB)
                    emit_finalize(prevB)

                # ======== phase B(t) ========
                cpB = prevB["c"] if t else None
                emit_sigma(curB, 0)
                emit_tanh(curA, 1)
                emit_h(curA, 1, last=last)
                emit_tanh(curA, 0)
                emit_h(curA, 0, last=last)
                emit_chain(curB, cpB, 0)
                emit_sigma(curB, 1)
                emit_chain(curB, cpB, 1)
                # hA(t) fully written -> dense-A(t) + A-gates(t+1)
                emit_dense_pe(curA)
                nxtA = None
                if not last:
                    nxtA = new_state(0, t + 1)
                    emit_gates_pe(nxtA, curA["h8"])
                emit_finalize(curA)

                prevA, prevB = curA, curB
                curA = nxtA

            # ---- tail: B(steps-1) tanh/h, dense, finalize ------------------
            emit_tanh(prevB, 0)
            emit_h(prevB, 0, last=True)
            emit_tanh(prevB, 1)
            emit_h(prevB, 1, last=True)
            emit_dense_pe(prevB)
            emit_finalize(prevB)

    nc.compile()
    _prog_cache[steps] = nc
    return nc


def _prep_inputs(inputs, W_ih, W_hh, b_ih, b_hh, W_d, b_d):
    """Host-side prep: fuse dense feedback, chunk-permute, quantize."""
    import ml_dtypes

    U, I = UNITS, INPUT_DIM
    W_ih = np.asarray(W_ih, np.float64)
    W_hh = np.asarray(W_hh, np.float64)
    W_d = np.asarray(W_d, np.float64)
    b_ih = np.asarray(b_ih, np.float64)
    b_hh = np.asarray(b_hh, np.float64)
    b_d = np.asarray(b_d, np.float64)

    W_comb = W_hh + W_ih @ W_d              # [4U, U]
    b_comb = b_ih + b_hh + W_ih @ b_d       # [4U]
    bx = b_ih + b_hh                        # step-0 bias
    db = (bx - b_comb) / S                  # step-0 correction (ones-row)

    # chunk permutation: chunk m=(u,q) <- original gate block q*U + u*128
    perm = np.empty(4 * U, dtype=np.int64)
    for u in range(NU):
        for q in range(4):
            m = u * 4 + q
            src = q * U + u * 128
            perm[m * 128:(m + 1) * 128] = np.arange(src, src + 128)
    Wp = W_comb[perm].astype(np.float64)    # [4U(chunked), U]
    bp = b_comb[perm].copy()
    Wxp = W_ih[perm].copy()                 # [4U, I]
    dbp = db[perm].copy()
    for u in range(NU):                     # double g-gate rows (q==2)
        m = u * 4 + 2
        sl = slice(m * 128, (m + 1) * 128)
        Wp[sl] *= 2.0
        bp[sl] *= 2.0
        Wxp[sl] *= 2.0
        dbp[sl] *= 2.0

    # fp8 DR weights [128, j, i, 4U]: wh[p,j,i,m] = Wp[m, (2j+i)*128+p]/SW
    wh = np.empty((128, NJ, 2, 4 * U), dtype=np.float64)
    for j in range(NJ):
        for i2 in range(2):
            k = 2 * j + i2
            wh[:, j, i2, :] = Wp[:, k * 128:(k + 1) * 128].T / SW
    wh8 = wh.astype(np.float32).astype(ml_dtypes.float8_e4m3)

    # x-path fp16 (pre-divided by S) + bias-correction ones-row
    wxf = np.zeros((I + 1, 4 * U), dtype=np.float64)
    wxf[:I] = (Wxp / S).T
    wxf[I] = dbp
    wx16 = wxf.astype(np.float32).astype(np.float16)

    # rank-4 fp8 DoubleRow bias rows: plane0 scale 160, plane1 scale 10
    wb8 = np.zeros((4, NU, 2, 128), dtype=np.float64)
    binj = bp / S
    for u in range(NU):
        for q in range(4):
            m = u * 4 + q
            vals = binj[m * 128:(m + 1) * 128]
            hi = (vals / IND_HI).astype(np.float32).astype(
                ml_dtypes.float8_e4m3).astype(np.float64)
            lo = ((vals - hi * IND_HI) / IND_LO).astype(np.float32)
            wb8[q, u, 0] = hi
            wb8[q, u, 1] = lo
    wb8 = wb8.astype(np.float32).astype(ml_dtypes.float8_e4m3)

    ind = np.zeros((4, 2, 4 * SL), dtype=np.float32)
    for q in range(4):
        ind[q, 0, q * SL:(q + 1) * SL] = IND_HI
        ind[q, 1, q * SL:(q + 1) * SL] = IND_LO
    ind8 = ind.astype(ml_dtypes.float8_e4m3)

    wd = np.zeros((128, NU, 128), dtype=np.float32)
    wd[:, :, :I] = W_d.T.reshape(NU, 128, I).transpose(1, 0, 2)
    wd16 = wd.astype(np.float16)
    bdv = np.zeros((128, 1), dtype=np.float32)
    bdv[:I, 0] = b_d

    x_last = np.asarray(inputs[:, -1, :], dtype=np.float32)  # [B, I]
    in_maps = []
    for c in range(NCORES):
        x0 = np.zeros((I + 1, 2, SL), dtype=np.float32)
        blk = x_last[c * BL:(c + 1) * BL].T   # [I, BL]
        x0[:I, 0] = blk[:, :SL]
        x0[:I, 1] = blk[:, SL:]
        x0[I] = 1.0
        in_maps.append({
            "wh": wh8, "wx": wx16, "wb": wb8, "ind": ind8,
            "wd": wd16, "bd": bdv, "x0": x0.astype(np.float16),
        })
    return in_maps


def kernel(inputs, W_ih, W_hh, b_ih, b_hh, W_d, b_d):
    in_maps = _prep_inputs(np.asarray(inputs), W_ih, W_hh, b_ih, b_hh, W_d, b_d)
    nc = _build_program()
    res = run_bass_kernel_spmd(nc, in_maps, core_ids=list(range(NCORES)))
    parts = [np.transpose(res.results[c]["out"], (2, 0, 1)) for c in range(NCORES)]
    return np.ascontiguousarray(np.concatenate(parts, axis=0))


# revision 13
# speedup vs baseline: 1.9385x; 1.0074x over previous
"""Trainium2 Bass kernel for nn_AutoRegressive (LSTM cell, 64 autoregressive steps).

Strategy
--------
Data-parallel over batch: B=2048 split across 8 NeuronCores (256 rows each),
params replicated. Feature-major tiles: [feature_partition, batch_free].

The linear autoregressive feedback folds into the recurrence:
    gates_t = (W_hh + W_ih W_d) h_{t-1} + (b_ih + b_hh + W_ih b_d)
so only the fused recurrent matmul + output dense remain per step.

Speed structure (vs the bf16 baseline at 1.04ms):
1. fp8-e4m3 DoubleRow matmuls for the recurrence: each PE pass contracts TWO
   128-row K-slices at 0.5 cycles per moving row -> 4x the bf16 matmul rate.
   W scaled by 2^-11, h by 2^7 (powers of two, exact); the sigma activation's
   scale operand applies s = 2^-18 for free. The saturating LSTM dynamics
   absorb the fp8 noise (measured ~5e-3 rel err vs 3.9e-3 all-bf16).
2. Gate biases are pre-injected into PSUM by tiny rank-4 fp8-DoubleRow
   matmuls (chunk-indicator rhs at scales 160/10, hi+lo fp8 bias rows), so
   ONE merged Sigmoid instruction covers all 12 gate chunks of a 3-unit
   group; the g-gate is tanh(x) = 2 sigmoid(2x) - 1 with the 2x folded into
   the weights and the affine fixup done by a 4x-mode tensor_scalar on DVE.
3. All elementwise tensors are fp16 (not bf16): same 2-byte DVE perf modes
   (2x tensor_tensor / 4x tensor_scalar), 8x lower rounding noise; c stays
   fp32. The dense output matmul runs in fp16 (fp8 there fails the gate).
4. Two independent 128-row batch streams (A/B) are software-pipelined half a
   phase apart: each stream's serial tail (sigma -> c -> tanh -> h -> next
   matmul) hides under the other stream's activation block, so the Activation
   engine (the bottleneck at ~7.9us/step) never starves; tanh instructions
   are scheduled EARLY in the opposite phase so the h-chain feeding the next
   step's matmuls clears before the next sigma needs its PSUM tile. The PE
   never idles >3us (which would drop its p-state in the cost model).
"""

import sys

sys.path.insert(0, "/opt/trn_rl_repo")

import numpy as np

import concourse.bacc as bacc
import concourse.mybir as mybir
import concourse.tile as tile
from concourse.bass_utils import run_bass_kernel_spmd

UNITS = 768
INPUT_DIM = 96
OUT_STEPS = 64
NCORES = 8
B = 2048
BL = B // NCORES          # 256 batch rows per core
SL = BL // 2              # 128 rows per stream
NU = UNITS // 128         # 6 unit tiles
UG = 3                    # units per group
NG = NU // UG             # 2 groups per stream
NJ = NU // 2              # 3 K-pairs for DoubleRow
F32 = mybir.dt.float32
F16 = mybir.dt.float16
BF16 = mybir.dt.bfloat16
F8 = mybir.dt.float8e4
Sigmoid = mybir.ActivationFunctionType.Sigmoid
Tanh = mybir.ActivationFunctionType.Tanh
MULT = mybir.AluOpType.mult
ADD = mybir.AluOpType.add
SUB = mybir.AluOpType.subtract
DR = mybir.MatmulPerfMode.DoubleRow

SW = 2.0 ** -11           # fp8 weight scale
SH = 2.0 ** -7            # fp8 h scale (h8 stores h*128)
S = SW * SH               # sigma scale operand
IND_HI, IND_LO = 160.0, 10.0   # fp8 bias indicator scales

_prog_cache = {}


def _build_program(steps=OUT_STEPS):
    if steps in _prog_cache:
        return _prog_cache[steps]

    nc = bacc.Bacc("TRN2", target_bir_lowering=False, debug=False, num_devices=NCORES)
    wh_ext = nc.declare_dram_parameter("wh", [128, NJ, 2, 4 * UNITS], F8, isOutput=False)
    wx_ext = nc.declare_dram_parameter("wx", [INPUT_DIM + 1, 4 * UNITS], F16, isOutput=False)
    wb_ext = nc.declare_dram_parameter("wb", [4, NU, 2, 128], F8, isOutput=False)
    ind_ext = nc.declare_dram_parameter("ind", [4, 2, 4 * SL], F8, isOutput=False)
    wd_ext = nc.declare_dram_parameter("wd", [128, NU, 128], F16, isOutput=False)
    bd_ext = nc.declare_dram_parameter("bd", [128, 1], F32, isOutput=False)
    x0_ext = nc.declare_dram_parameter("x0", [INPUT_DIM + 1, 2, SL], F16, isOutput=False)
    out_ext = nc.declare_dram_parameter("out", [steps, INPUT_DIM, BL], F32, isOutput=True)

    with tile.TileContext(nc) as tc:
        with (
            tc.tile_pool(name="const", bufs=1) as const,
            tc.tile_pool(name="state", bufs=2) as state,
            tc.tile_pool(name="work", bufs=2) as work,
            tc.tile_pool(name="ps", bufs=2, space="PSUM") as ps,
        ):
            # ---- constants (DMA priority order) ----------------------------
            x0 = const.tile([INPUT_DIM + 1, 2, SL], F16, tag="x0")
            nc.sync.dma_start(x0[:], x0_ext[:])
            wx = const.tile([INPUT_DIM + 1, 4 * UNITS], F16, tag="wx")
            nc.sync.dma_start(wx[:], wx_ext[:])
            ind = const.tile([4, 2, 4 * SL], F8, tag="ind")
            nc.sync.dma_start(ind[:], ind_ext[:])
            wb = const.tile([4, NU, 2, 128], F8, tag="wb")
            nc.sync.dma_start(wb[:], wb_ext[:])
            bd = const.tile([128, 1], F32, tag="bd")
            nc.sync.dma_start(bd[:], bd_ext[:])
            wh = const.tile([128, NJ, 2, 4 * UNITS], F8, tag="wh")
            for j in (1, 2, 0):
                nc.sync.dma_start(wh[:, j], wh_ext[:, j])
            wd = const.tile([128, NU, 128], F16, tag="wd")
            nc.sync.dma_start(wd[:], wd_ext[:])

            # ramp the PE to its warm p-state on throwaway matmuls (x0 is the
            # first DMA to land, so warm-up starts earliest using it)
            warm = ps.tile([128, 4 * UG * SL], F32, tag="g", name="warm")
            for _ in range(16):
                nc.tensor.matmul(
                    warm[:, :256], x0[:, 0, :], x0[:],
                    start=True, stop=True, skip_group_check=True,
                )

            def new_state(ph, t):
                sfx = f"{'AB'[ph]}{t}"
                return {
                    "t": t, "ph": ph,
                    "gates": [None] * NG,
                    "ps": [None] * NG,
                    "h": state.tile([128, NU, SL], F16, tag=f"h{ph}", name=f"h{sfx}"),
                    "h8": state.tile([128, NU, SL], F8, tag=f"h8{ph}", name=f"h8{sfx}"),
                    "c": state.tile([128, NU, SL], F32, tag=f"c{ph}", name=f"c{sfx}"),
                    "tct": work.tile([128, NU, SL], F16, tag=f"tct{ph}", name=f"tct{sfx}"),
                    "ps_pred": None,
                }

            def emit_gates_pe(st, h8_prev):
                """PE work for one stream's step-t gate tiles, emitted
                per-group [bias, j0, j1, j2] so a stalled bias never blocks
                another group's passes."""
                ph, t = st["ph"], st["t"]
                for g in range(NG):
                    psg = ps.tile(
                        [128, UG, 4, SL], F32, tag="g", name=f"g{'AB'[ph]}{t}_{g}"
                    )
                    st["ps"][g] = psg
                    # rank-4 fp8 DoubleRow bias seed (start=True); one per
                    # unit-local so each matmul output stays in one PSUM bank
                    for uloc in range(UG):
                        u = UG * g + uloc
                        nc.tensor.matmul(
                            psg[:, uloc], wb[:, u], ind[:],
                            start=True, stop=False, perf_mode=DR,
                            skip_group_check=True,
                        )
                    if t == 0:
                        for uloc in range(UG):
                            u = UG * g + uloc
                            for q in range(4):
                                m = (u * 4 + q) * 128
                                nc.tensor.matmul(
                                    psg[:, uloc, q, :], wx[:, m:m + 128], x0[:, ph],
                                    start=False, stop=(uloc == UG - 1 and q == 3),
                                    skip_group_check=True,
                                )
                    else:
                        for j in (1, 2, 0):
                            for uloc in range(UG):
                                u = UG * g + uloc
                                for q in range(4):
                                    m = (u * 4 + q) * 128
                                    nc.tensor.matmul(
                                        psg[:, uloc, q, :],
                                        wh[:, j, :, m:m + 128],
                                        h8_prev[:, 2 * j:2 * j + 2, :],
                                        start=False, stop=(j == 0),
                                        perf_mode=DR, skip_group_check=True,
                                    )

            def emit_dense_pe(st, korder=None):
                ph, t = st["ph"], st["t"]
                ps_pred = ps.tile([128, SL], F32, tag="d", bufs=2, name=f"p{'AB'[ph]}{t}")
                st["ps_pred"] = ps_pred
                korder = korder or range(NU)
                for n, k in enumerate(korder):
                    nc.tensor.matmul(
                        ps_pred[:], wd[:, k], st["h"][:, k],
                        start=(n == 0), stop=(n == NU - 1),
                    )

            def emit_sigma(st, g):
                ph, t = st["ph"], st["t"]
                gates = work.tile(
                    [128, UG, 4, SL], F16, tag=f"gt{ph}", bufs=4,
                    name=f"gt{'AB'[ph]}{t}_{g}",
                )
                st["gates"][g] = gates
                nc.scalar.activation(gates[:], st["ps"][g][:], Sigmoid, scale=S)
                st["ps"][g] = None

            def emit_chain(st, c_prev, g):
                """Pool: f*c (parallel with DVE g-fix/i*g); c-sum on Pool for
                g0, on DVE for g1 (the last group's sum is on the critical
                tanh path, and DVE is faster per element)."""
                ph, t = st["ph"], st["t"]
                gates = st["gates"][g]
                i_ = gates[:, :, 0, :]
                f_ = gates[:, :, 1, :]
                gp = gates[:, :, 2, :]
                nc.vector.tensor_scalar(gp, gp, 2.0, 1.0, MULT, SUB)
                m1 = work.tile([128, UG, SL], F16, tag=f"m1{ph}", name=f"m1{'AB'[ph]}{t}_{g}")
                nc.vector.tensor_tensor(m1[:], i_, gp, MULT)
                cs = slice(UG * g, UG * (g + 1))
                if t == 0:
                    nc.gpsimd.tensor_scalar_add(st["c"][:, cs], m1[:], 0.0)
                else:
                    fc = work.tile([128, UG, SL], F32, tag=f"fc{ph}", name=f"fc{'AB'[ph]}{t}_{g}")
                    tail = t == OUT_STEPS - 1 and ph == 1
                    feng = nc.vector if tail else nc.gpsimd
                    feng.tensor_tensor(fc[:], f_, c_prev[:, cs], MULT)
                    eng = nc.vector if (g == NG - 1 or tail) else nc.gpsimd
                    eng.tensor_tensor(st["c"][:, cs], fc[:], m1[:], ADD)

            def emit_tanh(st, g):
                ts = slice(UG * g, UG * (g + 1))
                nc.scalar.activation(st["tct"][:, ts], st["c"][:, ts], Tanh)

            def emit_h(st, g, last=False):
                cs = slice(UG * g, UG * (g + 1))
                nc.vector.tensor_tensor(
                    st["h"][:, cs], st["gates"][g][:, :, 3, :], st["tct"][:, cs], MULT
                )
                if not last:
                    nc.vector.tensor_scalar(st["h8"][:, cs], st["h"][:, cs], 128.0, None, MULT)

            def emit_finalize(st):
                ph, t = st["ph"], st["t"]
                pred = work.tile([INPUT_DIM, SL], F32, tag=f"pr{ph}", name=f"pr{'AB'[ph]}{t}")
                # GPSIMD has no PSUM port -> this one runs on DVE
                nc.vector.tensor_scalar_add(pred[:], st["ps_pred"][:INPUT_DIM, :], bd[:INPUT_DIM])
                nc.sync.dma_start(out_ext[t, :, ph * SL:(ph + 1) * SL], pred[:])

            # ---- prologue: stream A step-0 gates ---------------------------
            curA = new_state(0, 0)
            emit_gates_pe(curA, None)
            prevA = prevB = None

            # ---- main loop -------------------------------------------------
            # Act steady order: [sA0 TB0' TB1' sA1 | sB0 TA0 TA1 sB1]; each
            # stream's tanh pair runs EARLY in the opposite phase so its
            # h-chain clears before the next sigma needs the PSUM tile. PE
            # blocks (gates/dense) are emitted AFTER the h-writes they read.
            for t in range(steps):
                last = t == steps - 1
                # ======== phase A(t) ========
                cpA = prevA["c"] if t else None
                emit_sigma(curA, 0)
                if t:
                    emit_tanh(prevB, 1)
                    emit_h(prevB, 1)
                    emit_tanh(prevB, 0)
                    emit_h(prevB, 0)
                emit_chain(curA, cpA, 0)
                emit_sigma(curA, 1)
                emit_chain(curA, cpA, 1)
                # hB(t-1) fully written in program order -> B-gates(t)+dense
                curB = new_state(1, t)
                emit_gates_pe(curB, prevB["h8"] if t else None)
                if t:
                    emit_dense_pe(prevB)
                    emit_finalize(prevB)

                # ======== phase B(t) ========
                cpB = prevB["c"] if t else None
                emit_sigma(curB, 0)
                emit_tanh(curA, 1)
                emit_h(curA, 1, last=last)
                emit_tanh(curA, 0)
                emit_h(curA, 0, last=last)
                emit_chain(curB, cpB, 0)
                emit_sigma(curB, 1)
                emit_chain(curB, cpB, 1)
                # hA(t) fully written -> dense-A(t) + A-gates(t+1)
                emit_dense_pe(curA)
                nxtA = None
                if not last:
                    nxtA = new_state(0, t + 1)
                    emit_gates_pe(nxtA, curA["h8"])
                emit_finalize(curA)

                prevA, prevB = curA, curB
                curA = nxtA

            # ---- tail: B(steps-1) tanh/h, dense, finalize ------------------
            # g1 first (its chain finishes last -> start its tanh asap), and
            # the dense contracts g1's k-slices first so it can begin before
            # hB-g0 lands.
            emit_tanh(prevB, 1)
            emit_h(prevB, 1, last=True)
            emit_tanh(prevB, 0)
            emit_h(prevB, 0, last=True)
            emit_dense_pe(prevB, korder=(3, 4, 5, 0, 1, 2))
            emit_finalize(prevB)

    nc.compile()
    _prog_cache[steps] = nc
    return nc


def _prep_inputs(inputs, W_ih, W_hh, b_ih, b_hh, W_d, b_d):
    """Host-side prep: fuse dense feedback, chunk-permute, quantize."""
    import ml_dtypes

    U, I = UNITS, INPUT_DIM
    W_ih = np.asarray(W_ih, np.float64)
    W_hh = np.asarray(W_hh, np.float64)
    W_d = np.asarray(W_d, np.float64)
    b_ih = np.asarray(b_ih, np.float64)
    b_hh = np.asarray(b_hh, np.float64)
    b_d = np.asarray(b_d, np.float64)

    W_comb = W_hh + W_ih @ W_d              # [4U, U]
    b_comb = b_ih + b_hh + W_ih @ b_d       # [4U]
    bx = b_ih + b_hh                        # step-0 bias
    db = (bx - b_comb) / S                  # step-0 correction (ones-row)

    # chunk permutation: chunk m=(u,q) <- original gate block q*U + u*128
    perm = np.empty(4 * U, dtype=np.int64)
    for u in range(NU):
        for q in range(4):
            m = u * 4 + q
            src = q * U + u * 128
            perm[m * 128:(m + 1) * 128] = np.arange(src, src + 128)
    Wp = W_comb[perm].astype(np.float64)    # [4U(chunked), U]
    bp = b_comb[perm].copy()
    Wxp = W_ih[perm].copy()                 # [4U, I]
    dbp = db[perm].copy()
    for u in range(NU):                     # double g-gate rows (q==2)
        m = u * 4 + 2
        sl = slice(m * 128, (m + 1) * 128)
        Wp[sl] *= 2.0
        bp[sl] *= 2.0
        Wxp[sl] *= 2.0
        dbp[sl] *= 2.0

    # fp8 DR weights [128, j, i, 4U]: wh[p,j,i,m] = Wp[m, (2j+i)*128+p]/SW
    wh = np.empty((128, NJ, 2, 4 * U), dtype=np.float64)
    for j in range(NJ):
        for i2 in range(2):
            k = 2 * j + i2
            wh[:, j, i2, :] = Wp[:, k * 128:(k + 1) * 128].T / SW
    wh8 = wh.astype(np.float32).astype(ml_dtypes.float8_e4m3)

    # x-path fp16 (pre-divided by S) + bias-correction ones-row
    wxf = np.zeros((I + 1, 4 * U), dtype=np.float64)
    wxf[:I] = (Wxp / S).T
    wxf[I] = dbp
    wx16 = wxf.astype(np.float32).astype(np.float16)

    # rank-4 fp8 DoubleRow bias rows: plane0 scale 160, plane1 scale 10
    wb8 = np.zeros((4, NU, 2, 128), dtype=np.float64)
    binj = bp / S
    for u in range(NU):
        for q in range(4):
            m = u * 4 + q
            vals = binj[m * 128:(m + 1) * 128]
            hi = (vals / IND_HI).astype(np.float32).astype(
                ml_dtypes.float8_e4m3).astype(np.float64)
            lo = ((vals - hi * IND_HI) / IND_LO).astype(np.float32)
            wb8[q, u, 0] = hi
            wb8[q, u, 1] = lo
    wb8 = wb8.astype(np.float32).astype(ml_dtypes.float8_e4m3)

    ind = np.zeros((4, 2, 4 * SL), dtype=np.float32)
    for q in range(4):
        ind[q, 0, q * SL:(q + 1) * SL] = IND_HI
        ind[q, 1, q * SL:(q + 1) * SL] = IND_LO
    ind8 = ind.astype(ml_dtypes.float8_e4m3)

    wd = np.zeros((128, NU, 128), dtype=np.float32)
    wd[:, :, :I] = W_d.T.reshape(NU, 128, I).transpose(1, 0, 2)
    wd16 = wd.astype(np.float16)
    bdv = np.zeros((128, 1), dtype=np.float32)
    bdv[:I, 0] = b_d

    x_last = np.asarray(inputs[:, -1, :], dtype=np.float32)  # [B, I]
    in_maps = []
    for c in range(NCORES):
        x0 = np.zeros((I + 1, 2, SL), dtype=np.float32)
        blk = x_last[c * BL:(c + 1) * BL].T   # [I, BL]
        x0[:I, 0] = blk[:, :SL]
        x0[:I, 1] = blk[:, SL:]
        x0[I] = 1.0
        in_maps.append({
            "wh": wh8, "wx": wx16, "wb": wb8, "ind": ind8,
            "wd": wd16, "bd": bdv, "x0": x0.astype(np.float16),
        })
    return in_maps


def kernel(inputs, W_ih, W_hh, b_ih, b_hh, W_d, b_d):
    in_maps = _prep_inputs(np.asarray(inputs), W_ih, W_hh, b_ih, b_hh, W_d, b_d)
    nc = _build_program()
    res = run_bass_kernel_spmd(nc, in_maps, core_ids=list(range(NCORES)))
    parts = [np.transpose(res.results[c]["out"], (2, 0, 1)) for c in range(NCORES)]
    return np.ascontiguousarray(np.concatenate(parts, axis=0))


# revision 14
# speedup vs baseline: 1.9388x; 1.0002x over previous
"""Trainium2 Bass kernel for nn_AutoRegressive (LSTM cell, 64 autoregressive steps).

Strategy
--------
Data-parallel over batch: B=2048 split across 8 NeuronCores (256 rows each),
params replicated. Feature-major tiles: [feature_partition, batch_free].

The linear autoregressive feedback folds into the recurrence:
    gates_t = (W_hh + W_ih W_d) h_{t-1} + (b_ih + b_hh + W_ih b_d)
so only the fused recurrent matmul + output dense remain per step.

Speed structure (vs the bf16 baseline at 1.04ms):
1. fp8-e4m3 DoubleRow matmuls for the recurrence: each PE pass contracts TWO
   128-row K-slices at 0.5 cycles per moving row -> 4x the bf16 matmul rate.
   W scaled by 2^-11, h by 2^7 (powers of two, exact); the sigma activation's
   scale operand applies s = 2^-18 for free. The saturating LSTM dynamics
   absorb the fp8 noise (measured ~5e-3 rel err vs 3.9e-3 all-bf16).
2. Gate biases are pre-injected into PSUM by tiny rank-4 fp8-DoubleRow
   matmuls (chunk-indicator rhs at scales 160/10, hi+lo fp8 bias rows), so
   ONE merged Sigmoid instruction covers all 12 gate chunks of a 3-unit
   group; the g-gate is tanh(x) = 2 sigmoid(2x) - 1 with the 2x folded into
   the weights and the affine fixup done by a 4x-mode tensor_scalar on DVE.
3. All elementwise tensors are fp16 (not bf16): same 2-byte DVE perf modes
   (2x tensor_tensor / 4x tensor_scalar), 8x lower rounding noise; c stays
   fp32. The dense output matmul runs in fp16 (fp8 there fails the gate).
4. Two independent 128-row batch streams (A/B) are software-pipelined half a
   phase apart: each stream's serial tail (sigma -> c -> tanh -> h -> next
   matmul) hides under the other stream's activation block, so the Activation
   engine (the bottleneck at ~7.9us/step) never starves; tanh instructions
   are scheduled EARLY in the opposite phase so the h-chain feeding the next
   step's matmuls clears before the next sigma needs its PSUM tile. The PE
   never idles >3us (which would drop its p-state in the cost model).
"""

import sys

sys.path.insert(0, "/opt/trn_rl_repo")

import numpy as np

import concourse.bacc as bacc
import concourse.mybir as mybir
import concourse.tile as tile
from concourse.bass_utils import run_bass_kernel_spmd

UNITS = 768
INPUT_DIM = 96
OUT_STEPS = 64
NCORES = 8
B = 2048
BL = B // NCORES          # 256 batch rows per core
SL = BL // 2              # 128 rows per stream
NU = UNITS // 128         # 6 unit tiles
UG = 3                    # units per group
NG = NU // UG             # 2 groups per stream
NJ = NU // 2              # 3 K-pairs for DoubleRow
F32 = mybir.dt.float32
F16 = mybir.dt.float16
BF16 = mybir.dt.bfloat16
F8 = mybir.dt.float8e4
Sigmoid = mybir.ActivationFunctionType.Sigmoid
Tanh = mybir.ActivationFunctionType.Tanh
MULT = mybir.AluOpType.mult
ADD = mybir.AluOpType.add
SUB = mybir.AluOpType.subtract
DR = mybir.MatmulPerfMode.DoubleRow

SW = 2.0 ** -11           # fp8 weight scale
SH = 2.0 ** -7            # fp8 h scale (h8 stores h*128)
S = SW * SH               # sigma scale operand
IND_HI, IND_LO = 160.0, 10.0   # fp8 bias indicator scales

_prog_cache = {}


def _build_program(steps=OUT_STEPS):
    if steps in _prog_cache:
        return _prog_cache[steps]

    nc = bacc.Bacc("TRN2", target_bir_lowering=False, debug=False, num_devices=NCORES)
    wh_ext = nc.declare_dram_parameter("wh", [128, NJ, 2, 4 * UNITS], F8, isOutput=False)
    wx_ext = nc.declare_dram_parameter("wx", [INPUT_DIM + 1, 4 * UNITS], F16, isOutput=False)
    wb_ext = nc.declare_dram_parameter("wb", [4, NU, 2, 128], F8, isOutput=False)
    ind_ext = nc.declare_dram_parameter("ind", [4, 2, 4 * SL], F8, isOutput=False)
    wd_ext = nc.declare_dram_parameter("wd", [128, NU, 128], F16, isOutput=False)
    bd_ext = nc.declare_dram_parameter("bd", [128, 1], F32, isOutput=False)
    x0_ext = nc.declare_dram_parameter("x0", [INPUT_DIM + 1, 2, SL], F16, isOutput=False)
    out_ext = nc.declare_dram_parameter("out", [steps, INPUT_DIM, BL], F32, isOutput=True)

    with tile.TileContext(nc) as tc:
        with (
            tc.tile_pool(name="const", bufs=1) as const,
            tc.tile_pool(name="state", bufs=2) as state,
            tc.tile_pool(name="work", bufs=2) as work,
            tc.tile_pool(name="ps", bufs=2, space="PSUM") as ps,
        ):
            # ---- constants (DMA priority order) ----------------------------
            x0 = const.tile([INPUT_DIM + 1, 2, SL], F16, tag="x0")
            nc.sync.dma_start(x0[:], x0_ext[:])
            wx = const.tile([INPUT_DIM + 1, 4 * UNITS], F16, tag="wx")
            nc.sync.dma_start(wx[:], wx_ext[:])
            ind = const.tile([4, 2, 4 * SL], F8, tag="ind")
            nc.sync.dma_start(ind[:], ind_ext[:])
            wb = const.tile([4, NU, 2, 128], F8, tag="wb")
            nc.sync.dma_start(wb[:], wb_ext[:])
            bd = const.tile([128, 1], F32, tag="bd")
            nc.sync.dma_start(bd[:], bd_ext[:])
            wh = const.tile([128, NJ, 2, 4 * UNITS], F8, tag="wh")
            for j in (1, 2, 0):
                nc.sync.dma_start(wh[:, j], wh_ext[:, j])
            wd = const.tile([128, NU, 128], F16, tag="wd")
            nc.sync.dma_start(wd[:], wd_ext[:])

            # ramp the PE to its warm p-state on throwaway matmuls (x0 is the
            # first DMA to land, so warm-up starts earliest using it)
            warm = ps.tile([128, 4 * UG * SL], F32, tag="g", name="warm")
            for _ in range(12):
                nc.tensor.matmul(
                    warm[:, :256], x0[:, 0, :], x0[:],
                    start=True, stop=True, skip_group_check=True,
                )

            def new_state(ph, t):
                sfx = f"{'AB'[ph]}{t}"
                return {
                    "t": t, "ph": ph,
                    "gates": [None] * NG,
                    "ps": [None] * NG,
                    "h": state.tile([128, NU, SL], F16, tag=f"h{ph}", name=f"h{sfx}"),
                    "h8": state.tile([128, NU, SL], F8, tag=f"h8{ph}", name=f"h8{sfx}"),
                    "c": state.tile([128, NU, SL], F32, tag=f"c{ph}", name=f"c{sfx}"),
                    "tct": work.tile([128, NU, SL], F16, tag=f"tct{ph}", name=f"tct{sfx}"),
                    "ps_pred": None,
                }

            def emit_gates_pe(st, h8_prev):
                """PE work for one stream's step-t gate tiles, emitted
                per-group [bias, j0, j1, j2] so a stalled bias never blocks
                another group's passes."""
                ph, t = st["ph"], st["t"]
                for g in range(NG):
                    psg = ps.tile(
                        [128, UG, 4, SL], F32, tag="g", name=f"g{'AB'[ph]}{t}_{g}"
                    )
                    st["ps"][g] = psg
                    # rank-4 fp8 DoubleRow bias seed (start=True); one per
                    # unit-local so each matmul output stays in one PSUM bank
                    for uloc in range(UG):
                        u = UG * g + uloc
                        nc.tensor.matmul(
                            psg[:, uloc], wb[:, u], ind[:],
                            start=True, stop=False, perf_mode=DR,
                            skip_group_check=True,
                        )
                    if t == 0:
                        for uloc in range(UG):
                            u = UG * g + uloc
                            for q in range(4):
                                m = (u * 4 + q) * 128
                                nc.tensor.matmul(
                                    psg[:, uloc, q, :], wx[:, m:m + 128], x0[:, ph],
                                    start=False, stop=(uloc == UG - 1 and q == 3),
                                    skip_group_check=True,
                                )
                    else:
                        for j in (1, 2, 0):
                            for uloc in range(UG):
                                u = UG * g + uloc
                                for q in range(4):
                                    m = (u * 4 + q) * 128
                                    nc.tensor.matmul(
                                        psg[:, uloc, q, :],
                                        wh[:, j, :, m:m + 128],
                                        h8_prev[:, 2 * j:2 * j + 2, :],
                                        start=False, stop=(j == 0),
                                        perf_mode=DR, skip_group_check=True,
                                    )

            def emit_dense_pe(st, korder=None):
                ph, t = st["ph"], st["t"]
                ps_pred = ps.tile([128, SL], F32, tag="d", bufs=2, name=f"p{'AB'[ph]}{t}")
                st["ps_pred"] = ps_pred
                korder = korder or range(NU)
                for n, k in enumerate(korder):
                    nc.tensor.matmul(
                        ps_pred[:], wd[:, k], st["h"][:, k],
                        start=(n == 0), stop=(n == NU - 1),
                    )

            def emit_sigma(st, g):
                ph, t = st["ph"], st["t"]
                gates = work.tile(
                    [128, UG, 4, SL], F16, tag=f"gt{ph}", bufs=4,
                    name=f"gt{'AB'[ph]}{t}_{g}",
                )
                st["gates"][g] = gates
                nc.scalar.activation(gates[:], st["ps"][g][:], Sigmoid, scale=S)
                st["ps"][g] = None

            def emit_chain(st, c_prev, g):
                """Pool: f*c (parallel with DVE g-fix/i*g); c-sum on Pool for
                g0, on DVE for g1 (the last group's sum is on the critical
                tanh path, and DVE is faster per element)."""
                ph, t = st["ph"], st["t"]
                gates = st["gates"][g]
                i_ = gates[:, :, 0, :]
                f_ = gates[:, :, 1, :]
                gp = gates[:, :, 2, :]
                nc.vector.tensor_scalar(gp, gp, 2.0, 1.0, MULT, SUB)
                m1 = work.tile([128, UG, SL], F16, tag=f"m1{ph}", name=f"m1{'AB'[ph]}{t}_{g}")
                nc.vector.tensor_tensor(m1[:], i_, gp, MULT)
                cs = slice(UG * g, UG * (g + 1))
                if t == 0:
                    nc.vector.tensor_scalar_add(st["c"][:, cs], m1[:], 0.0)
                else:
                    fc = work.tile([128, UG, SL], F32, tag=f"fc{ph}", name=f"fc{'AB'[ph]}{t}_{g}")
                    tail = t == OUT_STEPS - 1 and ph == 1
                    feng = nc.vector if tail else nc.gpsimd
                    feng.tensor_tensor(fc[:], f_, c_prev[:, cs], MULT)
                    eng = nc.vector if (g == NG - 1 or tail) else nc.gpsimd
                    eng.tensor_tensor(st["c"][:, cs], fc[:], m1[:], ADD)

            def emit_tanh(st, g):
                ts = slice(UG * g, UG * (g + 1))
                nc.scalar.activation(st["tct"][:, ts], st["c"][:, ts], Tanh)

            def emit_h(st, g, last=False):
                cs = slice(UG * g, UG * (g + 1))
                nc.vector.tensor_tensor(
                    st["h"][:, cs], st["gates"][g][:, :, 3, :], st["tct"][:, cs], MULT
                )
                if not last:
                    nc.vector.tensor_scalar(st["h8"][:, cs], st["h"][:, cs], 128.0, None, MULT)

            def emit_finalize(st):
                ph, t = st["ph"], st["t"]
                pred = work.tile([INPUT_DIM, SL], F32, tag=f"pr{ph}", name=f"pr{'AB'[ph]}{t}")
                # GPSIMD has no PSUM port -> this one runs on DVE
                nc.vector.tensor_scalar_add(pred[:], st["ps_pred"][:INPUT_DIM, :], bd[:INPUT_DIM])
                nc.sync.dma_start(out_ext[t, :, ph * SL:(ph + 1) * SL], pred[:])

            # ---- prologue: stream A step-0 gates ---------------------------
            curA = new_state(0, 0)
            emit_gates_pe(curA, None)
            prevA = prevB = None

            # ---- main loop -------------------------------------------------
            # Act steady order: [sA0 TB0' TB1' sA1 | sB0 TA0 TA1 sB1]; each
            # stream's tanh pair runs EARLY in the opposite phase so its
            # h-chain clears before the next sigma needs the PSUM tile. PE
            # blocks (gates/dense) are emitted AFTER the h-writes they read.
            for t in range(steps):
                last = t == steps - 1
                # ======== phase A(t) ========
                cpA = prevA["c"] if t else None
                emit_sigma(curA, 0)
                if t:
                    emit_tanh(prevB, 1)
                    emit_h(prevB, 1)
                    emit_tanh(prevB, 0)
                    emit_h(prevB, 0)
                emit_chain(curA, cpA, 0)
                emit_sigma(curA, 1)
                emit_chain(curA, cpA, 1)
                # hB(t-1) fully written in program order -> B-gates(t)+dense
                curB = new_state(1, t)
                emit_gates_pe(curB, prevB["h8"] if t else None)
                if t:
                    emit_dense_pe(prevB)
                    emit_finalize(prevB)

                # ======== phase B(t) ========
                cpB = prevB["c"] if t else None
                emit_sigma(curB, 0)
                emit_tanh(curA, 1)
                emit_h(curA, 1, last=last)
                emit_tanh(curA, 0)
                emit_h(curA, 0, last=last)
                emit_chain(curB, cpB, 0)
                emit_sigma(curB, 1)
                emit_chain(curB, cpB, 1)
                # hA(t) fully written -> dense-A(t) + A-gates(t+1)
                emit_dense_pe(curA)
                nxtA = None
                if not last:
                    nxtA = new_state(0, t + 1)
                    emit_gates_pe(nxtA, curA["h8"])
                emit_finalize(curA)

                prevA, prevB = curA, curB
                curA = nxtA

            # ---- tail: B(steps-1) tanh/h, dense, finalize ------------------
            # g1 first (its chain finishes last -> start its tanh asap), and
            # the dense contracts g1's k-slices first so it can begin before
            # hB-g0 lands.
            emit_tanh(prevB, 1)
            emit_h(prevB, 1, last=True)
            emit_tanh(prevB, 0)
            emit_h(prevB, 0, last=True)
            emit_dense_pe(prevB, korder=(3, 4, 5, 0, 1, 2))
            emit_finalize(prevB)

    nc.compile()
    _prog_cache[steps] = nc
    return nc


def _prep_inputs(inputs, W_ih, W_hh, b_ih, b_hh, W_d, b_d):
    """Host-side prep: fuse dense feedback, chunk-permute, quantize."""
    import ml_dtypes

    U, I = UNITS, INPUT_DIM
    W_ih = np.asarray(W_ih, np.float64)
    W_hh = np.asarray(W_hh, np.float64)
    W_d = np.asarray(W_d, np.float64)
    b_ih = np.asarray(b_ih, np.float64)
    b_hh = np.asarray(b_hh, np.float64)
    b_d = np.asarray(b_d, np.float64)

    W_comb = W_hh + W_ih @ W_d              # [4U, U]
    b_comb = b_ih + b_hh + W_ih @ b_d       # [4U]
    bx = b_ih + b_hh                        # step-0 bias
    db = (bx - b_comb) / S                  # step-0 correction (ones-row)

    # chunk permutation: chunk m=(u,q) <- original gate block q*U + u*128
    perm = np.empty(4 * U, dtype=np.int64)
    for u in range(NU):
        for q in range(4):
            m = u * 4 + q
            src = q * U + u * 128
            perm[m * 128:(m + 1) * 128] = np.arange(src, src + 128)
    Wp = W_comb[perm].astype(np.float64)    # [4U(chunked), U]
    bp = b_comb[perm].copy()
    Wxp = W_ih[perm].copy()                 # [4U, I]
    dbp = db[perm].copy()
    for u in range(NU):                     # double g-gate rows (q==2)
        m = u * 4 + 2
        sl = slice(m * 128, (m + 1) * 128)
        Wp[sl] *= 2.0
        bp[sl] *= 2.0
        Wxp[sl] *= 2.0
        dbp[sl] *= 2.0

    # fp8 DR weights [128, j, i, 4U]: wh[p,j,i,m] = Wp[m, (2j+i)*128+p]/SW
    wh = np.empty((128, NJ, 2, 4 * U), dtype=np.float64)
    for j in range(NJ):
        for i2 in range(2):
            k = 2 * j + i2
            wh[:, j, i2, :] = Wp[:, k * 128:(k + 1) * 128].T / SW
    wh8 = wh.astype(np.float32).astype(ml_dtypes.float8_e4m3)

    # x-path fp16 (pre-divided by S) + bias-correction ones-row
    wxf = np.zeros((I + 1, 4 * U), dtype=np.float64)
    wxf[:I] = (Wxp / S).T
    wxf[I] = dbp
    wx16 = wxf.astype(np.float32).astype(np.float16)

    # rank-4 fp8 DoubleRow bias rows: plane0 scale 160, plane1 scale 10
    wb8 = np.zeros((4, NU, 2, 128), dtype=np.float64)
    binj = bp / S
    for u in range(NU):
        for q in range(4):
            m = u * 4 + q
            vals = binj[m * 128:(m + 1) * 128]
            hi = (vals / IND_HI).astype(np.float32).astype(
                ml_dtypes.float8_e4m3).astype(np.float64)
            lo = ((vals - hi * IND_HI) / IND_LO).astype(np.float32)
            wb8[q, u, 0] = hi
            wb8[q, u, 1] = lo
    wb8 = wb8.astype(np.float32).astype(ml_dtypes.float8_e4m3)

    ind = np.zeros((4, 2, 4 * SL), dtype=np.float32)
    for q in range(4):
        ind[q, 0, q * SL:(q + 1) * SL] = IND_HI
        ind[q, 1, q * SL:(q + 1) * SL] = IND_LO
    ind8 = ind.astype(ml_dtypes.float8_e4m3)

    wd = np.zeros((128, NU, 128), dtype=np.float32)
    wd[:, :, :I] = W_d.T.reshape(NU, 128, I).transpose(1, 0, 2)
    wd16 = wd.astype(np.float16)
    bdv = np.zeros((128, 1), dtype=np.float32)
    bdv[:I, 0] = b_d

    x_last = np.asarray(inputs[:, -1, :], dtype=np.float32)  # [B, I]
    in_maps = []
    for c in range(NCORES):
        x0 = np.zeros((I + 1, 2, SL), dtype=np.float32)
        blk = x_last[c * BL:(c + 1) * BL].T   # [I, BL]
        x0[:I, 0] = blk[:, :SL]
        x0[:I, 1] = blk[:, SL:]
        x0[I] = 1.0
        in_maps.append({
            "wh": wh8, "wx": wx16, "wb": wb8, "ind": ind8,
            "wd": wd16, "bd": bdv, "x0": x0.astype(np.float16),
        })
    return in_maps


def kernel(inputs, W_ih, W_hh, b_ih, b_hh, W_d, b_d):
    in_maps = _prep_inputs(np.asarray(inputs), W_ih, W_hh, b_ih, b_hh, W_d, b_d)
    nc = _build_program()
    res = run_bass_kernel_spmd(nc, in_maps, core_ids=list(range(NCORES)))
    parts = [np.transpose(res.results[c]["out"], (2, 0, 1)) for c in range(NCORES)]
    return np.ascontiguousarray(np.concatenate(parts, axis=0))
